# revision 1
# baseline (speedup 1.0000x reference)
"""Trainium2 Bass kernel for the DifferentiableCBFLayer batched dual-FISTA QP.

Strategy (pure data parallel, 8 cores x 4096 samples):
  Each core holds its 4096 samples as [128 partitions, 32 free] f32 "planes"
  (one plane per scalar quantity). The 26-row constraint system is reduced to
  25 rows (the all-zero "extra" row provably contributes nothing) with row
  order [obs x10, nei x7, cn, slack x3, box x4]. Only the 36 entries of
  columns 0,1 for the 18 geometric rows are per-sample; columns 2-4 are
  per-row constants (+- sqrt(Pinv_c) * mask), which lets both matvec
  directions run as a handful of large multi-plane DVE instructions:

    forward  x~ = Qadd + SS * [colsum01(W o z), R2(z), R3(z), R4(z)]
    backward T_m = W_m0*x0 + W_m1*x1 - x'_c(m)   (x'_c = scaled col sums)

  The FISTA iteration is run in a rescaled dual space (lam-hat = lam~ - b~,
  y-hat likewise; step folded into SS/b~) so one iteration is only:
    forward (8 DVE ops) + backward (9 DVE + 4 ACT ops)
    + arg=T+y (1) + lam' = max(arg,0)-b~ (fused STT, 1)
    + y' = (1+b)lam' - b lam  (fused LN_BWD_DX custom op, 1)

  The power iteration for L reuses the same forward/backward machinery with
  constant scale planes. All 330 iterations are fully unrolled (no loop
  back-edge cost); betas are host-precomputed fp32 constants.
"""
import os
from contextlib import ExitStack

import numpy as np

import concourse.bass as bass
import concourse.tile as tile
from concourse import mybir
from concourse.bass_utils import run_bass_kernel_spmd

f32 = mybir.dt.float32
AX = mybir.AxisListType
OP = mybir.AluOpType
AF = mybir.ActivationFunctionType

P = 128
F = 32
BPC = P * F            # samples per core
NCORES = 8
B_FULL = BPC * NCORES  # 32768

N_POWER = 30
N_FISTA = 300

MAX_OBS = 10
MAX_NEI = 7
BIG = 1000.0
PINV = np.array([0.5, 0.5, 1.0 / 200.0, 1.0 / 200.0, 1.0 / 200.0], np.float64)
K0 = float(np.float32(np.sqrt(PINV[0])))      # sqrt(1/2)
PINV2 = float(np.float32(PINV[2]))
SQ2 = float(np.float32(np.sqrt(2.0)))         # 2*K0 (= -q~ coefficient)

RAW_SPECS = [
    ("u_nominal", (BPC, 2)),
    ("v_current", (BPC, 1)),
    ("p_obs", (BPC, MAX_OBS, 2)),
    ("obs_mask", (BPC, MAX_OBS)),
    ("p_agents", (BPC, MAX_NEI, 2)),
    ("v_agents_local", (BPC, MAX_NEI, 2)),
    ("agents_mask", (BPC, MAX_NEI)),
    ("p_c_agent", (BPC, 1, 2)),
    ("v_c_agent", (BPC, 1, 2)),
    ("closest_mask", (BPC, 1)),
]
# All inputs are packed host-side into one (BPC, NFEAT) array: a single DMA
# means a single DMA-queue semaphore, keeping every instruction's sync-wait
# count below walrus's per-instruction limit.
NFEAT = 73
F_U, F_V, F_OBS, F_OM, F_AG, F_VA, F_AM, F_PC, F_VC, F_CM = 0, 2, 3, 23, 33, 47, 61, 68, 70, 72


def _betas(n):
    """Host fp32 replica of the on-device tk recursion."""
    one, half, four = np.float32(1.0), np.float32(0.5), np.float32(4.0)
    tk = np.float32(1.0)
    out = []
    for _ in range(n):
        tk1 = half * (one + np.sqrt(one + four * tk * tk, dtype=np.float32))
        beta = np.float32((tk - one) / tk1)
        out.append(float(beta))
        tk = tk1
    return out


# --------------------------------------------------------------------------
# emission helpers
# --------------------------------------------------------------------------
def _pl(t, i, n=1):
    """planes [i, i+n) of a plane-buffer tile as flat [P, n*F]."""
    return t[:, i * F:(i + n) * F]


def _pv(t, nplanes):
    """[P, nplanes, F] view of a plane-buffer tile."""
    return t[:].rearrange("p (m f) -> p m f", m=nplanes)


def _bc(plane_ap, n):
    """broadcast one [P, F] plane to [P, n, F] (step-0 middle dim)."""
    return plane_ap.unsqueeze(1).broadcast_to([P, n, F])


class Emit:
    def __init__(self, ctx, tc):
        self.tc = tc
        self.nc = tc.nc
        pool = ctx.enter_context(tc.tile_pool(name="state", bufs=1))
        self.fence_pool = ctx.enter_context(tc.tile_pool(name="fence", bufs=2))
        T = lambda n, tag: pool.tile([P, n * F], f32, name=tag, tag=tag)
        self.W = T(36, "W")          # Abar cols 0,1 in (row, col) pair order
        self.b = T(25, "b")          # unscaled h
        self.btil = T(25, "btil")    # sqrt(s) * b
        self.lamA = T(25, "lamA")
        self.lamB = T(25, "lamB")
        self.yh = T(25, "yh")        # y-hat (doubles as power-iteration v)
        self.Tbuf = T(25, "Tbuf")    # backward output (doubles as power w)
        self.prod = T(36, "prod")    # product scratch (also precompute scratch)
        self.X = T(5, "X")
        self.R = T(5, "R")
        self.SS = T(5, "SS")
        self.SSp = T(5, "SSp")
        self.Qadd = T(5, "Qadd")
        self.cs2 = T(2, "cs2")
        self.bx = T(2, "bx")
        self.u2 = T(2, "u2")
        self.sc1 = T(1, "sc1")       # small scalars-per-sample planes
        self.sc2 = T(1, "sc2")
        self.ns = T(1, "ns")
        self.rn = T(1, "rn")
        self.rs = T(1, "rs")
        self.sstar = T(1, "sstar")
        self.v2 = T(1, "v2")         # 2*v^2
        self.opack = T(2, "opack")

    def _act_fence(self, act_insts):
        """Tiny DVE memset carrying the sync-wait on ACT producers: walrus
        rejects instructions with >1 sync wait, and a DVE consumer of ACT
        output usually also needs its own-engine wait. The fence absorbs the
        ACT wait so the consumer keeps a single wait. Rotating 2-buf target
        keeps the fence's own WAW dep old enough to need no self-wait."""
        ft = self.fence_pool.tile([P, 1], f32, name="fence", tag="fence")
        ins = self.nc.vector.memset(ft[:], 0.0)
        for ai in act_insts:
            tile.add_dep_helper(ins.ins, ai.ins, sync=True, reason="act fence")
        return ins

    # ---------------- fwd/bwd machinery ----------------
    def forward(self, z, SSbuf, Qadd, X):
        nc = self.nc
        zv = _pv(z, 25)
        W4 = self.W[:].rearrange("p (r c f) -> p r c f", r=18, c=2)
        P4 = self.prod[:].rearrange("p (r c f) -> p r c f", r=18, c=2)
        # F1: per-sample products of cols 0,1 for the 18 geometric rows
        nc.vector.tensor_tensor(
            out=P4, in0=W4,
            in1=zv[:, 0:18].unsqueeze(2).broadcast_to([P, 18, 2, F]),
            op=OP.mult)
        # F2: column sums over the 18 rows  -> cs2 = [S0raw, S1raw]
        nc.vector.tensor_reduce(
            out=self.cs2[:],
            in_=self.prod[:].rearrange("p (r cf) -> p cf r", r=18),
            axis=AX.X, op=OP.add)
        # F3: box pair differences bx = [z22-z21, z24-z23]
        zbox = z[:, 21 * F:25 * F].rearrange("p (c g f) -> p c g f", c=2, g=2)
        nc.vector.tensor_tensor(
            out=self.bx[:].rearrange("p (c f) -> p c f", c=2),
            in0=zbox[:, :, 1, :], in1=zbox[:, :, 0, :], op=OP.subtract)
        # F4: R[0:2] = K0*bx + cs2
        nc.vector.scalar_tensor_tensor(
            out=_pl(self.R, 0, 2), in0=self.bx[:], scalar=K0, in1=self.cs2[:],
            op0=OP.mult, op1=OP.add)
        # F5/F6: R2 = sum(obs z)+z18 ; R3 = sum(nei z)+z19 ; R4 = z17+z20
        zf = z[:].rearrange("p (m f) -> p f m", m=25)
        nc.vector.tensor_reduce(out=_pl(self.R, 2), in_=zf[:, :, 0:10],
                                axis=AX.X, op=OP.add)
        nc.vector.tensor_tensor(out=_pl(self.R, 2), in0=_pl(self.R, 2),
                                in1=_pl(z, 18), op=OP.add)
        nc.vector.tensor_reduce(out=_pl(self.R, 3), in_=zf[:, :, 10:17],
                                axis=AX.X, op=OP.add)
        nc.vector.tensor_tensor(out=_pl(self.R, 3), in0=_pl(self.R, 3),
                                in1=_pl(z, 19), op=OP.add)
        nc.vector.tensor_tensor(out=_pl(self.R, 4), in0=_pl(z, 17),
                                in1=_pl(z, 20), op=OP.add)
        # F7/F8: X = SS*R (+ Qadd)
        nc.vector.tensor_tensor(out=X[:], in0=SSbuf[:], in1=self.R[:], op=OP.mult)
        if Qadd is not None:
            nc.vector.tensor_tensor(out=X[:], in0=X[:], in1=Qadd[:], op=OP.add)

    def backward_T(self, X):
        nc = self.nc
        Tb = self.Tbuf
        W4 = self.W[:].rearrange("p (r c f) -> p r c f", r=18, c=2)
        P4 = self.prod[:].rearrange("p (r c f) -> p r c f", r=18, c=2)
        x2 = X[:, 0:2 * F].rearrange("p (c f) -> p c f", c=2)
        # B1: products W[m,c] * x_c
        nc.vector.tensor_tensor(
            out=P4, in0=W4,
            in1=x2.unsqueeze(1).broadcast_to([P, 18, 2, F]), op=OP.mult)
        # B2: T[0:18] = pc0 + pc1
        nc.vector.tensor_tensor(
            out=_pl(Tb, 0, 18).rearrange("p (r f) -> p r f", r=18),
            in0=P4[:, :, 0, :], in1=P4[:, :, 1, :], op=OP.add)
        # B3-B5: subtract shared scaled-colsum planes
        nc.vector.tensor_tensor(
            out=_pl(Tb, 0, 10).rearrange("p (r f) -> p r f", r=10),
            in0=_pl(Tb, 0, 10).rearrange("p (r f) -> p r f", r=10),
            in1=_bc(_pl(self.X, 2), 10), op=OP.subtract)
        nc.vector.tensor_tensor(
            out=_pl(Tb, 10, 7).rearrange("p (r f) -> p r f", r=7),
            in0=_pl(Tb, 10, 7).rearrange("p (r f) -> p r f", r=7),
            in1=_bc(_pl(self.X, 3), 7), op=OP.subtract)
        nc.vector.tensor_tensor(out=_pl(Tb, 17), in0=_pl(Tb, 17),
                                in1=_pl(self.X, 4), op=OP.subtract)
        # B6 (ACT): T[slack] = -x'[2:5]
        a1 = nc.scalar.activation(_pl(Tb, 18, 3), _pl(self.X, 2, 3), AF.Copy, scale=-1.0)
        # B7-B9 (ACT): box rows +-K0*x0, +-K0*x1
        a2 = nc.scalar.activation(self.u2[:], _pl(self.X, 0, 2), AF.Copy, scale=-K0)
        tbox = Tb[:, 21 * F:25 * F].rearrange("p (c g f) -> p c g f", c=2, g=2)
        u2v = self.u2[:].rearrange("p (c f) -> p c f", c=2)
        a3 = nc.scalar.activation(tbox[:, :, 0, :], u2v, AF.Copy)
        a4 = nc.scalar.activation(tbox[:, :, 1, :], u2v, AF.Copy, scale=-1.0)
        self._act_fence([a1, a2, a3, a4])

    # ---------------- precompute ----------------
    def precompute(self, natt):
        nc = self.nc
        Wv = _pv(self.W, 36)
        bv = _pv(self.b, 25)
        STT = nc.vector.scalar_tensor_tensor
        TT = nc.vector.tensor_tensor

        # feature access patterns into the packed native tile
        pk = natt[:].rearrange("p (f a) -> p a f", a=NFEAT)
        self.pk = pk
        obs = pk[:, F_OBS:F_OBS + 20, :].rearrange("p (o c) f -> p o c f", c=2)
        lx, ly = obs[:, :, 0, :], obs[:, :, 1, :]
        om = pk[:, F_OM:F_OM + MAX_OBS, :]
        ag = pk[:, F_AG:F_AG + 14, :].rearrange("p (o c) f -> p o c f", c=2)
        ax, ay = ag[:, :, 0, :], ag[:, :, 1, :]
        va = pk[:, F_VA:F_VA + 14, :].rearrange("p (o c) f -> p o c f", c=2)
        vjx, vjy = va[:, :, 0, :], va[:, :, 1, :]
        am = pk[:, F_AM:F_AM + MAX_NEI, :]
        cx, cy = pk[:, F_PC, :], pk[:, F_PC + 1, :]
        cvx, cvy = pk[:, F_VC, :], pk[:, F_VC + 1, :]
        cm = pk[:, F_CM, :]
        v = pk[:, F_V, :]

        # v2 = 2*v^2
        STT(out=self.v2[:], in0=v, scalar=2.0, in1=v, op0=OP.mult, op1=OP.mult)

        sA = self.prod[:, 0:10 * F].rearrange("p (o f) -> p o f", o=10)
        sB = self.prod[:, 10 * F:20 * F].rearrange("p (o f) -> p o f", o=10)

        # ---- obs rows (planes 0-9; W pairs 0..19) ----
        W0 = Wv[:, 0:20].rearrange("p (o c) f -> p o c f", c=2)[:, :, 0, :]
        W1 = Wv[:, 0:20].rearrange("p (o c) f -> p o c f", c=2)[:, :, 1, :]
        STT(out=W0, in0=lx, scalar=2.0 * K0, in1=om, op0=OP.mult, op1=OP.mult)
        STT(out=sA, in0=ly, scalar=2.0 * K0, in1=_bc(v, 10), op0=OP.mult, op1=OP.mult)
        TT(out=W1, in0=sA, in1=om, op=OP.mult)
        # h_rhs = 2v^2 - 6 lx v + 2 lx^2 + 2 ly^2 - 0.5
        STT(out=sA, in0=lx, scalar=-6.0, in1=_bc(v, 10), op0=OP.mult, op1=OP.mult)
        STT(out=sB, in0=lx, scalar=2.0, in1=lx, op0=OP.mult, op1=OP.mult)
        TT(out=sA, in0=sA, in1=sB, op=OP.add)
        STT(out=sB, in0=ly, scalar=2.0, in1=ly, op0=OP.mult, op1=OP.mult)
        TT(out=sA, in0=sA, in1=sB, op=OP.add)
        TT(out=sA, in0=sA, in1=_bc(self.v2[:], 10), op=OP.add)
        nc.vector.tensor_scalar(out=sA, in0=sA, scalar1=-0.5, scalar2=None, op0=OP.add)
        # mask: b = (h - BIG)*m + BIG
        STT(out=sA, in0=sA, scalar=-BIG, in1=om, op0=OP.add, op1=OP.mult)
        nc.vector.tensor_scalar(out=bv[:, 0:10], in0=sA, scalar1=BIG, scalar2=None, op0=OP.add)

        # ---- nei rows (planes 10-16; W pairs 20..33) ----
        sA7 = self.prod[:, 0:7 * F].rearrange("p (o f) -> p o f", o=7)
        sB7 = self.prod[:, 7 * F:14 * F].rearrange("p (o f) -> p o f", o=7)
        sC7 = self.prod[:, 14 * F:21 * F].rearrange("p (o f) -> p o f", o=7)
        W0 = Wv[:, 20:34].rearrange("p (o c) f -> p o c f", c=2)[:, :, 0, :]
        W1 = Wv[:, 20:34].rearrange("p (o c) f -> p o c f", c=2)[:, :, 1, :]
        STT(out=W0, in0=ax, scalar=2.0 * K0, in1=am, op0=OP.mult, op1=OP.mult)
        STT(out=sA7, in0=ay, scalar=2.0 * K0, in1=_bc(v, 7), op0=OP.mult, op1=OP.mult)
        STT(out=sB7, in0=ay, scalar=-2.0 * K0, in1=vjx, op0=OP.mult, op1=OP.mult)
        TT(out=sA7, in0=sA7, in1=sB7, op=OP.add)
        STT(out=sB7, in0=ax, scalar=2.0 * K0, in1=vjy, op0=OP.mult, op1=OP.mult)
        TT(out=sA7, in0=sA7, in1=sB7, op=OP.add)
        TT(out=W1, in0=sA7, in1=am, op=OP.mult)
        # h = 2v^2 - 4 v vjx + 2 vjx^2 + 2 vjy^2 - 6 ax v + 6 ax vjx + 6 ay vjy
        #     + 2 ax^2 + 2 ay^2 - 1.28
        STT(out=sA7, in0=vjx, scalar=-4.0, in1=_bc(v, 7), op0=OP.mult, op1=OP.mult)
        STT(out=sB7, in0=vjx, scalar=2.0, in1=vjx, op0=OP.mult, op1=OP.mult)
        TT(out=sA7, in0=sA7, in1=sB7, op=OP.add)
        STT(out=sB7, in0=vjy, scalar=2.0, in1=vjy, op0=OP.mult, op1=OP.mult)
        TT(out=sA7, in0=sA7, in1=sB7, op=OP.add)
        STT(out=sB7, in0=ax, scalar=-6.0, in1=_bc(v, 7), op0=OP.mult, op1=OP.mult)
        TT(out=sA7, in0=sA7, in1=sB7, op=OP.add)
        STT(out=sB7, in0=ax, scalar=6.0, in1=vjx, op0=OP.mult, op1=OP.mult)
        TT(out=sA7, in0=sA7, in1=sB7, op=OP.add)
        STT(out=sB7, in0=ay, scalar=6.0, in1=vjy, op0=OP.mult, op1=OP.mult)
        TT(out=sA7, in0=sA7, in1=sB7, op=OP.add)
        STT(out=sB7, in0=ax, scalar=2.0, in1=ax, op0=OP.mult, op1=OP.mult)
        TT(out=sA7, in0=sA7, in1=sB7, op=OP.add)
        STT(out=sB7, in0=ay, scalar=2.0, in1=ay, op0=OP.mult, op1=OP.mult)
        TT(out=sA7, in0=sA7, in1=sB7, op=OP.add)
        TT(out=sA7, in0=sA7, in1=_bc(self.v2[:], 7), op=OP.add)
        nc.vector.tensor_scalar(out=sA7, in0=sA7, scalar1=-1.28, scalar2=None, op0=OP.add)
        STT(out=sA7, in0=sA7, scalar=-BIG, in1=am, op0=OP.add, op1=OP.mult)
        nc.vector.tensor_scalar(out=bv[:, 10:17], in0=sA7, scalar1=BIG, scalar2=None, op0=OP.add)

        # ---- cn row (plane 17; W pair 34,35) ----
        s1 = _pl(self.prod, 0)
        s2 = _pl(self.prod, 1)
        STT(out=Wv[:, 34], in0=cx, scalar=-2.0 * K0, in1=cm, op0=OP.mult, op1=OP.mult)
        STT(out=s1, in0=cy, scalar=-2.0 * K0, in1=v, op0=OP.mult, op1=OP.mult)
        STT(out=s2, in0=cy, scalar=2.0 * K0, in1=cvx, op0=OP.mult, op1=OP.mult)
        TT(out=s1, in0=s1, in1=s2, op=OP.add)
        STT(out=s2, in0=cx, scalar=-2.0 * K0, in1=cvy, op0=OP.mult, op1=OP.mult)
        TT(out=s1, in0=s1, in1=s2, op=OP.add)
        TT(out=Wv[:, 35], in0=s1, in1=cm, op=OP.mult)
        # h = -2v^2 + 4 v cvx - 2 cvx^2 - 2 cvy^2 + 6 cx v - 6 cx cvx - 6 cy cvy
        #     - 2 cx^2 - 2 cy^2 + 50
        STT(out=s1, in0=cvx, scalar=4.0, in1=v, op0=OP.mult, op1=OP.mult)
        STT(out=s2, in0=cvx, scalar=-2.0, in1=cvx, op0=OP.mult, op1=OP.mult)
        TT(out=s1, in0=s1, in1=s2, op=OP.add)
        STT(out=s2, in0=cvy, scalar=-2.0, in1=cvy, op0=OP.mult, op1=OP.mult)
        TT(out=s1, in0=s1, in1=s2, op=OP.add)
        STT(out=s2, in0=cx, scalar=6.0, in1=v, op0=OP.mult, op1=OP.mult)
        TT(out=s1, in0=s1, in1=s2, op=OP.add)
        STT(out=s2, in0=cx, scalar=-6.0, in1=cvx, op0=OP.mult, op1=OP.mult)
        TT(out=s1, in0=s1, in1=s2, op=OP.add)
        STT(out=s2, in0=cy, scalar=-6.0, in1=cvy, op0=OP.mult, op1=OP.mult)
        TT(out=s1, in0=s1, in1=s2, op=OP.add)
        STT(out=s2, in0=cx, scalar=-2.0, in1=cx, op0=OP.mult, op1=OP.mult)
        TT(out=s1, in0=s1, in1=s2, op=OP.add)
        STT(out=s2, in0=cy, scalar=-2.0, in1=cy, op0=OP.mult, op1=OP.mult)
        TT(out=s1, in0=s1, in1=s2, op=OP.add)
        TT(out=s1, in0=s1, in1=self.v2[:], op=OP.subtract)
        nc.vector.tensor_scalar(out=s1, in0=s1, scalar1=50.0, scalar2=None, op0=OP.add)
        STT(out=s1, in0=s1, scalar=-BIG, in1=cm, op0=OP.add, op1=OP.mult)
        nc.vector.tensor_scalar(out=_pl(self.b, 17), in0=s1, scalar1=BIG, scalar2=None, op0=OP.add)

        # ---- slack/box b, power scale planes ----
        nc.vector.memset(_pl(self.b, 18, 3), 0.0)
        nc.vector.memset(_pl(self.b, 21, 2), 2.0)
        nc.vector.memset(_pl(self.b, 23, 2), 1.0)
        nc.vector.memset(_pl(self.SSp, 0, 2), 1.0)
        for c in range(3):
            nc.vector.memset(_pl(self.SSp, 2 + c), -PINV2)


    def emit_rsqrt(self, dst, src, newton=0):
        """dst = rsqrt(src) via reciprocal + ACT Sqrt seed + Newton steps."""
        nc = self.nc
        nc.vector.reciprocal(out=self.sc1[:], in_=src)
        a = nc.scalar.activation(dst, self.sc1[:], AF.Sqrt)
        self._act_fence([a])
        for _ in range(newton):
            nc.vector.tensor_tensor(out=self.sc1[:], in0=dst, in1=dst, op=OP.mult)
            nc.vector.tensor_tensor(out=self.sc1[:], in0=src, in1=self.sc1[:], op=OP.mult)
            nc.vector.tensor_scalar(out=self.sc1[:], in0=self.sc1[:],
                                    scalar1=-0.5, scalar2=1.5, op0=OP.mult, op1=OP.add)
            nc.vector.tensor_tensor(out=dst, in0=dst, in1=self.sc1[:], op=OP.mult)

    # ---------------- power iteration + step ----------------
    def power_phase(self, n_power):
        nc = self.nc
        TT = nc.vector.tensor_tensor
        nc.vector.memset(self.yh[:], 1.0)
        sq = self.prod[:, 0:25 * F]
        for it in range(n_power):
            self.forward(self.yh, self.SSp, None, self.X)
            self.backward_T(self.X)
            # normalize: yh = w * rsqrt(sum w^2). Intermediate normalizations
            # only bound the range (direction is scale-invariant); the last one
            # enters the Rayleigh quotient, so refine it.
            TT(out=sq, in0=self.Tbuf[:], in1=self.Tbuf[:], op=OP.mult)
            nc.vector.tensor_reduce(
                out=self.ns[:], in_=sq.rearrange("p (m f) -> p f m", m=25),
                axis=AX.X, op=OP.add)
            self.emit_rsqrt(self.rn[:], self.ns[:],
                            newton=2 if it == n_power - 1 else 0)
            TT(out=_pv(self.yh, 25), in0=_pv(self.Tbuf, 25),
               in1=_bc(self.rn[:], 25), op=OP.mult)
        # Rayleigh L = v . (M v); then rs = rsqrt(L + 1e-6), s* = rs^2
        self.forward(self.yh, self.SSp, None, self.X)
        self.backward_T(self.X)
        TT(out=sq, in0=self.yh[:], in1=self.Tbuf[:], op=OP.mult)
        nc.vector.tensor_reduce(
            out=self.ns[:], in_=sq.rearrange("p (m f) -> p f m", m=25),
            axis=AX.X, op=OP.add)
        nc.vector.tensor_scalar(out=self.ns[:], in0=self.ns[:],
                                scalar1=1e-6, scalar2=None, op0=OP.add)
        self.emit_rsqrt(self.rs[:], self.ns[:], newton=2)
        TT(out=self.sstar[:], in0=self.rs[:], in1=self.rs[:], op=OP.mult)

    # ---------------- FISTA setup ----------------
    def fista_setup(self):
        nc = self.nc
        TT = nc.vector.tensor_tensor
        # btil = b * rs
        TT(out=_pv(self.btil, 25), in0=_pv(self.b, 25), in1=_bc(self.rs[:], 25),
           op=OP.mult)
        # SS = [-s*, -s*, PINV2*s* x3]
        a1 = nc.scalar.activation(
            _pl(self.SS, 0, 2).rearrange("p (c f) -> p c f", c=2),
            _bc(self.sstar[:], 2), AF.Copy, scale=-1.0)
        a2 = nc.scalar.activation(
            _pl(self.SS, 2, 3).rearrange("p (c f) -> p c f", c=3),
            _bc(self.sstar[:], 3), AF.Copy, scale=PINV2)
        self._act_fence([a1, a2])
        # Qadd = SS*FWD(btil) + rs * q~   (q~ = [sqrt2 u0, sqrt2 u1, 0,0,0])
        self.forward(self.btil, self.SS, None, self.Qadd)
        uap = self.pk[:, F_U:F_U + 2, :]
        nc.vector.scalar_tensor_tensor(
            out=self.u2[:].rearrange("p (c f) -> p c f", c=2),
            in0=uap, scalar=SQ2, in1=_bc(self.rs[:], 2), op0=OP.mult, op1=OP.mult)
        TT(out=_pl(self.Qadd, 0, 2), in0=_pl(self.Qadd, 0, 2), in1=self.u2[:],
           op=OP.add)
        # lam = yh = -btil
        a1 = nc.scalar.activation(self.lamA[:], self.btil[:], AF.Copy, scale=-1.0)
        a2 = nc.scalar.activation(self.yh[:], self.btil[:], AF.Copy, scale=-1.0)
        self._act_fence([a1, a2])

    # ---------------- FISTA loop ----------------
    def fista(self, n_fista):
        nc = self.nc
        betas = _betas(n_fista)
        lams = [self.lamA, self.lamB]
        for it in range(n_fista):
            lam_prev = lams[it % 2]
            lam_new = lams[(it + 1) % 2]
            beta = betas[it]
            self.forward(self.yh, self.SS, self.Qadd, self.X)
            self.backward_T(self.X)
            # arg = T + yh ; lam_new = max(arg,0) - btil
            nc.vector.tensor_tensor(out=self.Tbuf[:], in0=self.Tbuf[:],
                                    in1=self.yh[:], op=OP.add)
            nc.vector.scalar_tensor_tensor(
                out=lam_new[:], in0=self.Tbuf[:], scalar=0.0, in1=self.btil[:],
                op0=OP.max, op1=OP.subtract)
            # yh = lam_new + beta*(lam_new - lam_prev)
            nc.vector.tensor_tensor(out=self.yh[:], in0=lam_new[:],
                                    in1=lam_prev[:], op=OP.subtract)
            nc.vector.scalar_tensor_tensor(
                out=self.yh[:], in0=self.yh[:], scalar=float(beta),
                in1=lam_new[:], op0=OP.mult, op1=OP.add)
        return lams[n_fista % 2]

    # ---------------- finale ----------------
    def finale(self, lam_final, out_dram):
        nc = self.nc
        TT = nc.vector.tensor_tensor
        self.forward(lam_final, self.SS, self.Qadd, self.X)
        # u = K0 * X[0:2] / rs ; 1/rs = sqrt(L+1e-6) = ns * rs
        TT(out=self.sc1[:], in0=self.ns[:], in1=self.rs[:], op=OP.mult)
        a = nc.scalar.activation(self.sc2[:], self.sc1[:], AF.Copy, scale=K0)
        self.last_act = a
        self._act_fence([a])
        self.last_dve = TT(out=self.opack[:].rearrange("p (f c) -> p c f", c=2),
           in0=_pl(self.X, 0, 2).rearrange("p (c f) -> p c f", c=2),
           in1=_bc(self.sc2[:], 2), op=OP.mult)
        self.out_dma = nc.sync.dma_start(
            out=out_dram.ap().rearrange("(p f) c -> p (f c)", p=P),
            in_=self.opack[:])

    def terminals(self):
        return [self.in_dma, self.last_act, self.last_dve, self.out_dma]


def build_nc(n_power=N_POWER, n_fista=N_FISTA):
    nc = bass.Bass("TRN2")
    din = nc.dram_tensor("packed", [BPC, NFEAT], f32, kind="ExternalInput")
    dout = nc.dram_tensor("u_safe", [BPC, 2], f32, kind="ExternalOutput")

    with tile.TileContext(nc) as tc:
        with ExitStack() as ctx:
            em = Emit(ctx, tc)
            natpool = ctx.enter_context(tc.tile_pool(name="nat", bufs=1))
            natt = natpool.tile([P, F * NFEAT], f32, name="nat", tag="nat")
            # single SWDGE DMA -> one DMA semaphore for all downstream waits
            em.in_dma = nc.gpsimd.dma_start(
                out=natt[:], in_=din.ap().rearrange("(p f) a -> p (f a)", p=P))
            em.precompute(natt)
            em.power_phase(n_power)
            em.fista_setup()
            lam_final = em.fista(n_fista)
            em.finale(lam_final, dout)
            # Exit fence: the tile-exit drain would wait on every active proc
            # (ACT, DVE, DMA queues) at once, exceeding walrus's one-sync-wait
            # -per-instruction limit. Chain sync-engine NOPs, one dep each, so
            # the SP engine observes every proc before the drain.
            for ti in em.terminals():
                nop = nc.sync.nop()
                tile.add_dep_helper(nop.ins, ti.ins, sync=True,
                                    reason="exit fence")
    return nc


_NC_CACHE = {}


def _get_nc(n_power=N_POWER, n_fista=N_FISTA):
    key = (n_power, n_fista)
    if key not in _NC_CACHE:
        _NC_CACHE[key] = build_nc(n_power, n_fista)
    return _NC_CACHE[key]


def pack_inputs(inputs, lo, hi):
    """Pack the raw input dict (rows [lo, hi)) into one (n, NFEAT) array."""
    n = hi - lo
    cols = [np.asarray(inputs[name], np.float32)[lo:hi].reshape(n, -1)
            for name, _ in RAW_SPECS]
    return np.ascontiguousarray(np.concatenate(cols, axis=1))


def kernel(**inputs):
    """Full-input entry point: shard batch over 8 cores, run, gather."""
    nc = _get_nc()
    in_maps = [{"packed": pack_inputs(inputs, c * BPC, (c + 1) * BPC)}
               for c in range(NCORES)]
    res = run_bass_kernel_spmd(nc, in_maps, list(range(NCORES)))
    return np.concatenate([res.results[c]["u_safe"] for c in range(NCORES)],
                          axis=0)


if __name__ == "__main__":
    # smoke test on random data against a tiny numpy reference path
    rng = np.random.default_rng(0)
    demo = {
        "u_nominal": rng.standard_normal((B_FULL, 2)).astype(np.float32),
        "v_current": rng.uniform(0, 1, (B_FULL, 1)).astype(np.float32),
        "p_obs": (2 * rng.standard_normal((B_FULL, MAX_OBS, 2))).astype(np.float32),
        "obs_mask": np.ones((B_FULL, MAX_OBS), np.float32),
        "p_agents": (2 * rng.standard_normal((B_FULL, MAX_NEI, 2))).astype(np.float32),
        "v_agents_local": rng.standard_normal((B_FULL, MAX_NEI, 2)).astype(np.float32),
        "agents_mask": np.ones((B_FULL, MAX_NEI), np.float32),
        "p_c_agent": (2 * rng.standard_normal((B_FULL, 1, 2))).astype(np.float32),
        "v_c_agent": rng.standard_normal((B_FULL, 1, 2)).astype(np.float32),
        "closest_mask": np.ones((B_FULL, 1), np.float32),
    }
    out = kernel(**demo)
    print(out.shape, out.dtype, np.abs(out).max())



# revision 3
# speedup vs baseline: 1.0764x; 1.0764x over previous
"""Trainium2 Bass kernel for the DifferentiableCBFLayer batched dual-FISTA QP.

Strategy (pure data parallel, 8 cores x 4096 samples):
  Each core holds its 4096 samples as [128 partitions, 32 free] f32 "planes"
  (one plane per scalar quantity). The 26-row constraint system is reduced to
  25 rows (the all-zero "extra" row provably contributes nothing) with row
  order [obs x10, nei x7, cn, slack x3, box x4]. Only the 36 entries of
  columns 0,1 for the 18 geometric rows are per-sample; columns 2-4 are
  per-row constants (+- sqrt(Pinv_c) * mask), which lets both matvec
  directions run as a handful of large multi-plane DVE instructions:

    forward  x~ = Qadd + SS * [colsum01(W o z), R2(z), R3(z), R4(z)]
    backward T_m = W_m0*x0 + W_m1*x1 - x'_c(m)   (x'_c = scaled col sums)

  The FISTA iteration is run in a rescaled dual space (lam-hat = lam~ - b~,
  y-hat likewise; step folded into SS/b~) so one iteration is only:
    forward (8 DVE ops) + backward (9 DVE + 4 ACT ops)
    + arg=T+y (1) + lam' = max(arg,0)-b~ (fused STT, 1)
    + y' = (1+b)lam' - b lam  (fused LN_BWD_DX custom op, 1)

  The power iteration for L reuses the same forward/backward machinery with
  constant scale planes. All 330 iterations are fully unrolled (no loop
  back-edge cost); betas are host-precomputed fp32 constants.
"""
import os
from contextlib import ExitStack

import numpy as np

import concourse.bass as bass
import concourse.tile as tile
from concourse import mybir
from concourse.bass_utils import run_bass_kernel_spmd

f32 = mybir.dt.float32
AX = mybir.AxisListType
OP = mybir.AluOpType
AF = mybir.ActivationFunctionType

P = 128
F = 32
BPC = P * F            # samples per core
NCORES = 8
B_FULL = BPC * NCORES  # 32768

N_POWER = 30
N_FISTA = 300

MAX_OBS = 10
MAX_NEI = 7
BIG = 1000.0
PINV = np.array([0.5, 0.5, 1.0 / 200.0, 1.0 / 200.0, 1.0 / 200.0], np.float64)
K0 = float(np.float32(np.sqrt(PINV[0])))      # sqrt(1/2)
PINV2 = float(np.float32(PINV[2]))
SQ2 = float(np.float32(np.sqrt(2.0)))         # 2*K0 (= -q~ coefficient)

RAW_SPECS = [
    ("u_nominal", (BPC, 2)),
    ("v_current", (BPC, 1)),
    ("p_obs", (BPC, MAX_OBS, 2)),
    ("obs_mask", (BPC, MAX_OBS)),
    ("p_agents", (BPC, MAX_NEI, 2)),
    ("v_agents_local", (BPC, MAX_NEI, 2)),
    ("agents_mask", (BPC, MAX_NEI)),
    ("p_c_agent", (BPC, 1, 2)),
    ("v_c_agent", (BPC, 1, 2)),
    ("closest_mask", (BPC, 1)),
]
# All inputs are packed host-side into one (BPC, NFEAT) array: a single DMA
# means a single DMA-queue semaphore, keeping every instruction's sync-wait
# count below walrus's per-instruction limit.
NFEAT = 73
F_U, F_V, F_OBS, F_OM, F_AG, F_VA, F_AM, F_PC, F_VC, F_CM = 0, 2, 3, 23, 33, 47, 61, 68, 70, 72


def _betas(n):
    """Host fp32 replica of the on-device tk recursion."""
    one, half, four = np.float32(1.0), np.float32(0.5), np.float32(4.0)
    tk = np.float32(1.0)
    out = []
    for _ in range(n):
        tk1 = half * (one + np.sqrt(one + four * tk * tk, dtype=np.float32))
        beta = np.float32((tk - one) / tk1)
        out.append(float(beta))
        tk = tk1
    return out


# --------------------------------------------------------------------------
# emission helpers
# --------------------------------------------------------------------------
def _pl(t, i, n=1):
    """planes [i, i+n) of a plane-buffer tile as flat [P, n*F]."""
    return t[:, i * F:(i + n) * F]


def _pv(t, nplanes):
    """[P, nplanes, F] view of a plane-buffer tile."""
    return t[:].rearrange("p (m f) -> p m f", m=nplanes)


def _bc(plane_ap, n):
    """broadcast one [P, F] plane to [P, n, F] (step-0 middle dim)."""
    return plane_ap.unsqueeze(1).broadcast_to([P, n, F])


class Emit:
    def __init__(self, ctx, tc):
        self.tc = tc
        self.nc = tc.nc
        pool = ctx.enter_context(tc.tile_pool(name="state", bufs=1))
        self.fence_pool = ctx.enter_context(tc.tile_pool(name="fence", bufs=2))
        T = lambda n, tag: pool.tile([P, n * F], f32, name=tag, tag=tag)
        self.W = T(36, "W")          # Abar cols 0,1 in (row, col) pair order
        self.b = T(25, "b")          # unscaled h
        self.btil = T(25, "btil")    # sqrt(s) * b
        self.lamA = T(25, "lamA")
        self.lamB = T(25, "lamB")
        self.yh = T(25, "yh")        # y-hat (doubles as power-iteration v)
        self.Tbuf = T(25, "Tbuf")    # backward output (doubles as power w)
        self.prod = T(36, "prod")    # product scratch (also precompute scratch)
        self.X = T(5, "X")
        self.R = T(5, "R")
        self.SS = T(5, "SS")
        self.SSp = T(5, "SSp")
        self.Qadd = T(5, "Qadd")
        self.cs2 = T(2, "cs2")
        self.bx = T(2, "bx")
        self.u2 = T(2, "u2")
        self.sc1 = T(1, "sc1")       # small scalars-per-sample planes
        self.sc2 = T(1, "sc2")
        self.ns = T(1, "ns")
        self.rn = T(1, "rn")
        self.rs = T(1, "rs")
        self.sstar = T(1, "sstar")
        self.v2 = T(1, "v2")         # 2*v^2
        self.opack = T(2, "opack")
        self.tpA = T(25, "tpA")      # ACT-computed -beta*lam_prev (ping-pong)
        self.tpB = T(25, "tpB")

    def _act_fence(self, act_insts):
        """Tiny DVE memset carrying the sync-wait on ACT producers: walrus
        rejects instructions with >1 sync wait, and a DVE consumer of ACT
        output usually also needs its own-engine wait. The fence absorbs the
        ACT wait so the consumer keeps a single wait. Rotating 2-buf target
        keeps the fence's own WAW dep old enough to need no self-wait."""
        ft = self.fence_pool.tile([P, 1], f32, name="fence", tag="fence")
        ins = self.nc.vector.memset(ft[:], 0.0)
        for ai in act_insts:
            tile.add_dep_helper(ins.ins, ai.ins, sync=True, reason="act fence")
        return ins

    # ---------------- fwd/bwd machinery ----------------
    def forward(self, z, SSbuf, Qadd, X):
        nc = self.nc
        zv = _pv(z, 25)
        W4 = self.W[:].rearrange("p (r c f) -> p r c f", r=18, c=2)
        P4 = self.prod[:].rearrange("p (r c f) -> p r c f", r=18, c=2)
        # F1: per-sample products of cols 0,1 for the 18 geometric rows
        nc.vector.tensor_tensor(
            out=P4, in0=W4,
            in1=zv[:, 0:18].unsqueeze(2).broadcast_to([P, 18, 2, F]),
            op=OP.mult)
        # F2: column sums over the 18 rows  -> cs2 = [S0raw, S1raw]
        nc.vector.tensor_reduce(
            out=self.cs2[:],
            in_=self.prod[:].rearrange("p (r cf) -> p cf r", r=18),
            axis=AX.X, op=OP.add)
        # F3: box pair differences bx = [z22-z21, z24-z23]
        zbox = z[:, 21 * F:25 * F].rearrange("p (c g f) -> p c g f", c=2, g=2)
        nc.vector.tensor_tensor(
            out=self.bx[:].rearrange("p (c f) -> p c f", c=2),
            in0=zbox[:, :, 1, :], in1=zbox[:, :, 0, :], op=OP.subtract)
        # F4: R[0:2] = K0*bx + cs2
        nc.vector.scalar_tensor_tensor(
            out=_pl(self.R, 0, 2), in0=self.bx[:], scalar=K0, in1=self.cs2[:],
            op0=OP.mult, op1=OP.add)
        # F5/F6: R2 = sum(obs z)+z18 ; R3 = sum(nei z)+z19 ; R4 = z17+z20
        zf = z[:].rearrange("p (m f) -> p f m", m=25)
        nc.vector.tensor_reduce(out=_pl(self.R, 2), in_=zf[:, :, 0:10],
                                axis=AX.X, op=OP.add)
        nc.vector.tensor_tensor(out=_pl(self.R, 2), in0=_pl(self.R, 2),
                                in1=_pl(z, 18), op=OP.add)
        nc.vector.tensor_reduce(out=_pl(self.R, 3), in_=zf[:, :, 10:17],
                                axis=AX.X, op=OP.add)
        nc.vector.tensor_tensor(out=_pl(self.R, 3), in0=_pl(self.R, 3),
                                in1=_pl(z, 19), op=OP.add)
        nc.vector.tensor_tensor(out=_pl(self.R, 4), in0=_pl(z, 17),
                                in1=_pl(z, 20), op=OP.add)
        # F7/F8: X = SS*R (+ Qadd)
        nc.vector.tensor_tensor(out=X[:], in0=SSbuf[:], in1=self.R[:], op=OP.mult)
        if Qadd is not None:
            nc.vector.tensor_tensor(out=X[:], in0=X[:], in1=Qadd[:], op=OP.add)

    def backward_T(self, X):
        nc = self.nc
        Tb = self.Tbuf
        W4 = self.W[:].rearrange("p (r c f) -> p r c f", r=18, c=2)
        P4 = self.prod[:].rearrange("p (r c f) -> p r c f", r=18, c=2)
        x2 = X[:, 0:2 * F].rearrange("p (c f) -> p c f", c=2)
        # B1: products W[m,c] * x_c
        nc.vector.tensor_tensor(
            out=P4, in0=W4,
            in1=x2.unsqueeze(1).broadcast_to([P, 18, 2, F]), op=OP.mult)
        # B2: T[0:18] = pc0 + pc1
        nc.vector.tensor_tensor(
            out=_pl(Tb, 0, 18).rearrange("p (r f) -> p r f", r=18),
            in0=P4[:, :, 0, :], in1=P4[:, :, 1, :], op=OP.add)
        # B3-B5: subtract shared scaled-colsum planes
        nc.vector.tensor_tensor(
            out=_pl(Tb, 0, 10).rearrange("p (r f) -> p r f", r=10),
            in0=_pl(Tb, 0, 10).rearrange("p (r f) -> p r f", r=10),
            in1=_bc(_pl(self.X, 2), 10), op=OP.subtract)
        nc.vector.tensor_tensor(
            out=_pl(Tb, 10, 7).rearrange("p (r f) -> p r f", r=7),
            in0=_pl(Tb, 10, 7).rearrange("p (r f) -> p r f", r=7),
            in1=_bc(_pl(self.X, 3), 7), op=OP.subtract)
        nc.vector.tensor_tensor(out=_pl(Tb, 17), in0=_pl(Tb, 17),
                                in1=_pl(self.X, 4), op=OP.subtract)
        # B6 (ACT): T[slack] = -x'[2:5]
        a1 = nc.scalar.activation(_pl(Tb, 18, 3), _pl(self.X, 2, 3), AF.Copy, scale=-1.0)
        # B7-B9 (ACT): box rows +-K0*x0, +-K0*x1
        a2 = nc.scalar.activation(self.u2[:], _pl(self.X, 0, 2), AF.Copy, scale=-K0)
        tbox = Tb[:, 21 * F:25 * F].rearrange("p (c g f) -> p c g f", c=2, g=2)
        u2v = self.u2[:].rearrange("p (c f) -> p c f", c=2)
        a3 = nc.scalar.activation(tbox[:, :, 0, :], u2v, AF.Copy)
        a4 = nc.scalar.activation(tbox[:, :, 1, :], u2v, AF.Copy, scale=-1.0)
        self._act_fence([a1, a2, a3, a4])

    # ---------------- precompute ----------------
    def precompute(self, natt):
        nc = self.nc
        Wv = _pv(self.W, 36)
        bv = _pv(self.b, 25)
        STT = nc.vector.scalar_tensor_tensor
        TT = nc.vector.tensor_tensor

        # feature access patterns into the packed native tile
        pk = natt[:].rearrange("p (f a) -> p a f", a=NFEAT)
        self.pk = pk
        obs = pk[:, F_OBS:F_OBS + 20, :].rearrange("p (o c) f -> p o c f", c=2)
        lx, ly = obs[:, :, 0, :], obs[:, :, 1, :]
        om = pk[:, F_OM:F_OM + MAX_OBS, :]
        ag = pk[:, F_AG:F_AG + 14, :].rearrange("p (o c) f -> p o c f", c=2)
        ax, ay = ag[:, :, 0, :], ag[:, :, 1, :]
        va = pk[:, F_VA:F_VA + 14, :].rearrange("p (o c) f -> p o c f", c=2)
        vjx, vjy = va[:, :, 0, :], va[:, :, 1, :]
        am = pk[:, F_AM:F_AM + MAX_NEI, :]
        cx, cy = pk[:, F_PC, :], pk[:, F_PC + 1, :]
        cvx, cvy = pk[:, F_VC, :], pk[:, F_VC + 1, :]
        cm = pk[:, F_CM, :]
        v = pk[:, F_V, :]

        # v2 = 2*v^2
        STT(out=self.v2[:], in0=v, scalar=2.0, in1=v, op0=OP.mult, op1=OP.mult)

        sA = self.prod[:, 0:10 * F].rearrange("p (o f) -> p o f", o=10)
        sB = self.prod[:, 10 * F:20 * F].rearrange("p (o f) -> p o f", o=10)

        # ---- obs rows (planes 0-9; W pairs 0..19) ----
        W0 = Wv[:, 0:20].rearrange("p (o c) f -> p o c f", c=2)[:, :, 0, :]
        W1 = Wv[:, 0:20].rearrange("p (o c) f -> p o c f", c=2)[:, :, 1, :]
        STT(out=W0, in0=lx, scalar=2.0 * K0, in1=om, op0=OP.mult, op1=OP.mult)
        STT(out=sA, in0=ly, scalar=2.0 * K0, in1=_bc(v, 10), op0=OP.mult, op1=OP.mult)
        TT(out=W1, in0=sA, in1=om, op=OP.mult)
        # h_rhs = 2v^2 - 6 lx v + 2 lx^2 + 2 ly^2 - 0.5
        STT(out=sA, in0=lx, scalar=-6.0, in1=_bc(v, 10), op0=OP.mult, op1=OP.mult)
        STT(out=sB, in0=lx, scalar=2.0, in1=lx, op0=OP.mult, op1=OP.mult)
        TT(out=sA, in0=sA, in1=sB, op=OP.add)
        STT(out=sB, in0=ly, scalar=2.0, in1=ly, op0=OP.mult, op1=OP.mult)
        TT(out=sA, in0=sA, in1=sB, op=OP.add)
        TT(out=sA, in0=sA, in1=_bc(self.v2[:], 10), op=OP.add)
        nc.vector.tensor_scalar(out=sA, in0=sA, scalar1=-0.5, scalar2=None, op0=OP.add)
        # mask: b = (h - BIG)*m + BIG
        STT(out=sA, in0=sA, scalar=-BIG, in1=om, op0=OP.add, op1=OP.mult)
        nc.vector.tensor_scalar(out=bv[:, 0:10], in0=sA, scalar1=BIG, scalar2=None, op0=OP.add)

        # ---- nei rows (planes 10-16; W pairs 20..33) ----
        sA7 = self.prod[:, 0:7 * F].rearrange("p (o f) -> p o f", o=7)
        sB7 = self.prod[:, 7 * F:14 * F].rearrange("p (o f) -> p o f", o=7)
        sC7 = self.prod[:, 14 * F:21 * F].rearrange("p (o f) -> p o f", o=7)
        W0 = Wv[:, 20:34].rearrange("p (o c) f -> p o c f", c=2)[:, :, 0, :]
        W1 = Wv[:, 20:34].rearrange("p (o c) f -> p o c f", c=2)[:, :, 1, :]
        STT(out=W0, in0=ax, scalar=2.0 * K0, in1=am, op0=OP.mult, op1=OP.mult)
        STT(out=sA7, in0=ay, scalar=2.0 * K0, in1=_bc(v, 7), op0=OP.mult, op1=OP.mult)
        STT(out=sB7, in0=ay, scalar=-2.0 * K0, in1=vjx, op0=OP.mult, op1=OP.mult)
        TT(out=sA7, in0=sA7, in1=sB7, op=OP.add)
        STT(out=sB7, in0=ax, scalar=2.0 * K0, in1=vjy, op0=OP.mult, op1=OP.mult)
        TT(out=sA7, in0=sA7, in1=sB7, op=OP.add)
        TT(out=W1, in0=sA7, in1=am, op=OP.mult)
        # h = 2v^2 - 4 v vjx + 2 vjx^2 + 2 vjy^2 - 6 ax v + 6 ax vjx + 6 ay vjy
        #     + 2 ax^2 + 2 ay^2 - 1.28
        STT(out=sA7, in0=vjx, scalar=-4.0, in1=_bc(v, 7), op0=OP.mult, op1=OP.mult)
        STT(out=sB7, in0=vjx, scalar=2.0, in1=vjx, op0=OP.mult, op1=OP.mult)
        TT(out=sA7, in0=sA7, in1=sB7, op=OP.add)
        STT(out=sB7, in0=vjy, scalar=2.0, in1=vjy, op0=OP.mult, op1=OP.mult)
        TT(out=sA7, in0=sA7, in1=sB7, op=OP.add)
        STT(out=sB7, in0=ax, scalar=-6.0, in1=_bc(v, 7), op0=OP.mult, op1=OP.mult)
        TT(out=sA7, in0=sA7, in1=sB7, op=OP.add)
        STT(out=sB7, in0=ax, scalar=6.0, in1=vjx, op0=OP.mult, op1=OP.mult)
        TT(out=sA7, in0=sA7, in1=sB7, op=OP.add)
        STT(out=sB7, in0=ay, scalar=6.0, in1=vjy, op0=OP.mult, op1=OP.mult)
        TT(out=sA7, in0=sA7, in1=sB7, op=OP.add)
        STT(out=sB7, in0=ax, scalar=2.0, in1=ax, op0=OP.mult, op1=OP.mult)
        TT(out=sA7, in0=sA7, in1=sB7, op=OP.add)
        STT(out=sB7, in0=ay, scalar=2.0, in1=ay, op0=OP.mult, op1=OP.mult)
        TT(out=sA7, in0=sA7, in1=sB7, op=OP.add)
        TT(out=sA7, in0=sA7, in1=_bc(self.v2[:], 7), op=OP.add)
        nc.vector.tensor_scalar(out=sA7, in0=sA7, scalar1=-1.28, scalar2=None, op0=OP.add)
        STT(out=sA7, in0=sA7, scalar=-BIG, in1=am, op0=OP.add, op1=OP.mult)
        nc.vector.tensor_scalar(out=bv[:, 10:17], in0=sA7, scalar1=BIG, scalar2=None, op0=OP.add)

        # ---- cn row (plane 17; W pair 34,35) ----
        s1 = _pl(self.prod, 0)
        s2 = _pl(self.prod, 1)
        STT(out=Wv[:, 34], in0=cx, scalar=-2.0 * K0, in1=cm, op0=OP.mult, op1=OP.mult)
        STT(out=s1, in0=cy, scalar=-2.0 * K0, in1=v, op0=OP.mult, op1=OP.mult)
        STT(out=s2, in0=cy, scalar=2.0 * K0, in1=cvx, op0=OP.mult, op1=OP.mult)
        TT(out=s1, in0=s1, in1=s2, op=OP.add)
        STT(out=s2, in0=cx, scalar=-2.0 * K0, in1=cvy, op0=OP.mult, op1=OP.mult)
        TT(out=s1, in0=s1, in1=s2, op=OP.add)
        TT(out=Wv[:, 35], in0=s1, in1=cm, op=OP.mult)
        # h = -2v^2 + 4 v cvx - 2 cvx^2 - 2 cvy^2 + 6 cx v - 6 cx cvx - 6 cy cvy
        #     - 2 cx^2 - 2 cy^2 + 50
        STT(out=s1, in0=cvx, scalar=4.0, in1=v, op0=OP.mult, op1=OP.mult)
        STT(out=s2, in0=cvx, scalar=-2.0, in1=cvx, op0=OP.mult, op1=OP.mult)
        TT(out=s1, in0=s1, in1=s2, op=OP.add)
        STT(out=s2, in0=cvy, scalar=-2.0, in1=cvy, op0=OP.mult, op1=OP.mult)
        TT(out=s1, in0=s1, in1=s2, op=OP.add)
        STT(out=s2, in0=cx, scalar=6.0, in1=v, op0=OP.mult, op1=OP.mult)
        TT(out=s1, in0=s1, in1=s2, op=OP.add)
        STT(out=s2, in0=cx, scalar=-6.0, in1=cvx, op0=OP.mult, op1=OP.mult)
        TT(out=s1, in0=s1, in1=s2, op=OP.add)
        STT(out=s2, in0=cy, scalar=-6.0, in1=cvy, op0=OP.mult, op1=OP.mult)
        TT(out=s1, in0=s1, in1=s2, op=OP.add)
        STT(out=s2, in0=cx, scalar=-2.0, in1=cx, op0=OP.mult, op1=OP.mult)
        TT(out=s1, in0=s1, in1=s2, op=OP.add)
        STT(out=s2, in0=cy, scalar=-2.0, in1=cy, op0=OP.mult, op1=OP.mult)
        TT(out=s1, in0=s1, in1=s2, op=OP.add)
        TT(out=s1, in0=s1, in1=self.v2[:], op=OP.subtract)
        nc.vector.tensor_scalar(out=s1, in0=s1, scalar1=50.0, scalar2=None, op0=OP.add)
        STT(out=s1, in0=s1, scalar=-BIG, in1=cm, op0=OP.add, op1=OP.mult)
        nc.vector.tensor_scalar(out=_pl(self.b, 17), in0=s1, scalar1=BIG, scalar2=None, op0=OP.add)

        # ---- slack/box b, power scale planes ----
        nc.vector.memset(_pl(self.b, 18, 3), 0.0)
        nc.vector.memset(_pl(self.b, 21, 2), 2.0)
        nc.vector.memset(_pl(self.b, 23, 2), 1.0)
        nc.vector.memset(_pl(self.SSp, 0, 2), 1.0)
        for c in range(3):
            nc.vector.memset(_pl(self.SSp, 2 + c), -PINV2)


    def emit_rsqrt(self, dst, src, newton=0):
        """dst = rsqrt(src) via reciprocal + ACT Sqrt seed + Newton steps."""
        nc = self.nc
        nc.vector.reciprocal(out=self.sc1[:], in_=src)
        a = nc.scalar.activation(dst, self.sc1[:], AF.Sqrt)
        self._act_fence([a])
        for _ in range(newton):
            nc.vector.tensor_tensor(out=self.sc1[:], in0=dst, in1=dst, op=OP.mult)
            nc.vector.tensor_tensor(out=self.sc1[:], in0=src, in1=self.sc1[:], op=OP.mult)
            nc.vector.tensor_scalar(out=self.sc1[:], in0=self.sc1[:],
                                    scalar1=-0.5, scalar2=1.5, op0=OP.mult, op1=OP.add)
            nc.vector.tensor_tensor(out=dst, in0=dst, in1=self.sc1[:], op=OP.mult)

    # ---------------- power iteration + step ----------------
    def power_phase(self, n_power):
        nc = self.nc
        TT = nc.vector.tensor_tensor
        nc.vector.memset(self.yh[:], 1.0)
        sq = self.prod[:, 0:25 * F]
        for it in range(n_power):
            self.forward(self.yh, self.SSp, None, self.X)
            self.backward_T(self.X)
            # normalize: yh = w * rsqrt(sum w^2). Intermediate normalizations
            # only bound the range (direction is scale-invariant); the last one
            # enters the Rayleigh quotient, so refine it.
            TT(out=sq, in0=self.Tbuf[:], in1=self.Tbuf[:], op=OP.mult)
            nc.vector.tensor_reduce(
                out=self.ns[:], in_=sq.rearrange("p (m f) -> p f m", m=25),
                axis=AX.X, op=OP.add)
            self.emit_rsqrt(self.rn[:], self.ns[:],
                            newton=2 if it == n_power - 1 else 0)
            TT(out=_pv(self.yh, 25), in0=_pv(self.Tbuf, 25),
               in1=_bc(self.rn[:], 25), op=OP.mult)
        # Rayleigh L = v . (M v); then rs = rsqrt(L + 1e-6), s* = rs^2
        self.forward(self.yh, self.SSp, None, self.X)
        self.backward_T(self.X)
        TT(out=sq, in0=self.yh[:], in1=self.Tbuf[:], op=OP.mult)
        nc.vector.tensor_reduce(
            out=self.ns[:], in_=sq.rearrange("p (m f) -> p f m", m=25),
            axis=AX.X, op=OP.add)
        nc.vector.tensor_scalar(out=self.ns[:], in0=self.ns[:],
                                scalar1=1e-6, scalar2=None, op0=OP.add)
        self.emit_rsqrt(self.rs[:], self.ns[:], newton=2)
        TT(out=self.sstar[:], in0=self.rs[:], in1=self.rs[:], op=OP.mult)

    # ---------------- FISTA setup ----------------
    def fista_setup(self):
        nc = self.nc
        TT = nc.vector.tensor_tensor
        # btil = b * rs
        TT(out=_pv(self.btil, 25), in0=_pv(self.b, 25), in1=_bc(self.rs[:], 25),
           op=OP.mult)
        # SS = [-s*, -s*, PINV2*s* x3]
        a1 = nc.scalar.activation(
            _pl(self.SS, 0, 2).rearrange("p (c f) -> p c f", c=2),
            _bc(self.sstar[:], 2), AF.Copy, scale=-1.0)
        a2 = nc.scalar.activation(
            _pl(self.SS, 2, 3).rearrange("p (c f) -> p c f", c=3),
            _bc(self.sstar[:], 3), AF.Copy, scale=PINV2)
        self._act_fence([a1, a2])
        # Qadd = SS*FWD(btil) + rs * q~   (q~ = [sqrt2 u0, sqrt2 u1, 0,0,0])
        self.forward(self.btil, self.SS, None, self.Qadd)
        uap = self.pk[:, F_U:F_U + 2, :]
        nc.vector.scalar_tensor_tensor(
            out=self.u2[:].rearrange("p (c f) -> p c f", c=2),
            in0=uap, scalar=SQ2, in1=_bc(self.rs[:], 2), op0=OP.mult, op1=OP.mult)
        TT(out=_pl(self.Qadd, 0, 2), in0=_pl(self.Qadd, 0, 2), in1=self.u2[:],
           op=OP.add)
        # lam = yh = -btil
        a1 = nc.scalar.activation(self.lamA[:], self.btil[:], AF.Copy, scale=-1.0)
        a2 = nc.scalar.activation(self.yh[:], self.btil[:], AF.Copy, scale=-1.0)
        self._act_fence([a1, a2])

    # ---------------- FISTA loop ----------------
    def fista(self, n_fista):
        nc = self.nc
        betas = _betas(n_fista)
        lams = [self.lamA, self.lamB]
        # tp = -beta*lam_prev runs on the (otherwise idle) ACT engine. lam_prev
        # is ready at iteration start, so the ACT op overlaps the whole DVE
        # chain; the DVE only pays one fused STT for the momentum update:
        #   yh = (1+beta)*lam_new + tp
        # Rounding differs from (lam_new - lam_prev)*beta + lam_new by ~1 ulp,
        # which the harness tolerance absorbs.
        tps = [self.tpA, self.tpB]
        for it in range(n_fista):
            lam_prev = lams[it % 2]
            lam_new = lams[(it + 1) % 2]
            beta = betas[it]
            tp = tps[it % 2]
            a_tp = nc.scalar.activation(tp[:], lam_prev[:], AF.Copy,
                                        scale=-float(beta))
            self.forward(self.yh, self.SS, self.Qadd, self.X)
            self.backward_T(self.X)
            # arg = T + yh ; lam_new = max(arg,0) - btil
            nc.vector.tensor_tensor(out=self.Tbuf[:], in0=self.Tbuf[:],
                                    in1=self.yh[:], op=OP.add)
            nc.vector.scalar_tensor_tensor(
                out=lam_new[:], in0=self.Tbuf[:], scalar=0.0, in1=self.btil[:],
                op0=OP.max, op1=OP.subtract)
            # yh = (1+beta)*lam_new + tp
            self._act_fence([a_tp])
            nc.vector.scalar_tensor_tensor(
                out=self.yh[:], in0=lam_new[:], scalar=float(1.0 + beta),
                in1=tp[:], op0=OP.mult, op1=OP.add)
        return lams[n_fista % 2]

    # ---------------- finale ----------------
    def finale(self, lam_final, out_dram):
        nc = self.nc
        TT = nc.vector.tensor_tensor
        self.forward(lam_final, self.SS, self.Qadd, self.X)
        # u = K0 * X[0:2] / rs ; 1/rs = sqrt(L+1e-6) = ns * rs
        TT(out=self.sc1[:], in0=self.ns[:], in1=self.rs[:], op=OP.mult)
        a = nc.scalar.activation(self.sc2[:], self.sc1[:], AF.Copy, scale=K0)
        self.last_act = a
        self._act_fence([a])
        self.last_dve = TT(out=self.opack[:].rearrange("p (f c) -> p c f", c=2),
           in0=_pl(self.X, 0, 2).rearrange("p (c f) -> p c f", c=2),
           in1=_bc(self.sc2[:], 2), op=OP.mult)
        self.out_dma = nc.sync.dma_start(
            out=out_dram.ap().rearrange("(p f) c -> p (f c)", p=P),
            in_=self.opack[:])

    def terminals(self):
        return [self.in_dma, self.last_act, self.last_dve, self.out_dma]


def build_nc(n_power=N_POWER, n_fista=N_FISTA):
    nc = bass.Bass("TRN2")
    din = nc.dram_tensor("packed", [BPC, NFEAT], f32, kind="ExternalInput")
    dout = nc.dram_tensor("u_safe", [BPC, 2], f32, kind="ExternalOutput")

    with tile.TileContext(nc) as tc:
        with ExitStack() as ctx:
            em = Emit(ctx, tc)
            natpool = ctx.enter_context(tc.tile_pool(name="nat", bufs=1))
            natt = natpool.tile([P, F * NFEAT], f32, name="nat", tag="nat")
            # single SWDGE DMA -> one DMA semaphore for all downstream waits
            em.in_dma = nc.gpsimd.dma_start(
                out=natt[:], in_=din.ap().rearrange("(p f) a -> p (f a)", p=P))
            em.precompute(natt)
            em.power_phase(n_power)
            em.fista_setup()
            lam_final = em.fista(n_fista)
            em.finale(lam_final, dout)
            # Exit fence: the tile-exit drain would wait on every active proc
            # (ACT, DVE, DMA queues) at once, exceeding walrus's one-sync-wait
            # -per-instruction limit. Chain sync-engine NOPs, one dep each, so
            # the SP engine observes every proc before the drain.
            for ti in em.terminals():
                nop = nc.sync.nop()
                tile.add_dep_helper(nop.ins, ti.ins, sync=True,
                                    reason="exit fence")
    return nc


_NC_CACHE = {}


def _get_nc(n_power=N_POWER, n_fista=N_FISTA):
    key = (n_power, n_fista)
    if key not in _NC_CACHE:
        _NC_CACHE[key] = build_nc(n_power, n_fista)
    return _NC_CACHE[key]


def pack_inputs(inputs, lo, hi):
    """Pack the raw input dict (rows [lo, hi)) into one (n, NFEAT) array."""
    n = hi - lo
    cols = [np.asarray(inputs[name], np.float32)[lo:hi].reshape(n, -1)
            for name, _ in RAW_SPECS]
    return np.ascontiguousarray(np.concatenate(cols, axis=1))


def kernel(**inputs):
    """Full-input entry point: shard batch over 8 cores, run, gather."""
    nc = _get_nc()
    in_maps = [{"packed": pack_inputs(inputs, c * BPC, (c + 1) * BPC)}
               for c in range(NCORES)]
    res = run_bass_kernel_spmd(nc, in_maps, list(range(NCORES)))
    return np.concatenate([res.results[c]["u_safe"] for c in range(NCORES)],
                          axis=0)


if __name__ == "__main__":
    # smoke test on random data against a tiny numpy reference path
    rng = np.random.default_rng(0)
    demo = {
        "u_nominal": rng.standard_normal((B_FULL, 2)).astype(np.float32),
        "v_current": rng.uniform(0, 1, (B_FULL, 1)).astype(np.float32),
        "p_obs": (2 * rng.standard_normal((B_FULL, MAX_OBS, 2))).astype(np.float32),
        "obs_mask": np.ones((B_FULL, MAX_OBS), np.float32),
        "p_agents": (2 * rng.standard_normal((B_FULL, MAX_NEI, 2))).astype(np.float32),
        "v_agents_local": rng.standard_normal((B_FULL, MAX_NEI, 2)).astype(np.float32),
        "agents_mask": np.ones((B_FULL, MAX_NEI), np.float32),
        "p_c_agent": (2 * rng.standard_normal((B_FULL, 1, 2))).astype(np.float32),
        "v_c_agent": rng.standard_normal((B_FULL, 1, 2)).astype(np.float32),
        "closest_mask": np.ones((B_FULL, 1), np.float32),
    }
    out = kernel(**demo)
    print(out.shape, out.dtype, np.abs(out).max())



# revision 5
# speedup vs baseline: 1.1052x; 1.0268x over previous
"""Trainium2 Bass kernel for the DifferentiableCBFLayer batched dual-FISTA QP.

Strategy (pure data parallel, 8 cores x 4096 samples):
  Each core holds its 4096 samples as [128 partitions, 32 free] f32 "planes"
  (one plane per scalar quantity). The 26-row constraint system is reduced to
  25 rows (the all-zero "extra" row provably contributes nothing) with row
  order [obs x10, nei x7, cn, slack x3, box x4]. Only the 36 entries of
  columns 0,1 for the 18 geometric rows are per-sample; columns 2-4 are
  per-row constants (+- sqrt(Pinv_c) * mask), which lets both matvec
  directions run as a handful of large multi-plane DVE instructions:

    forward  x~ = Qadd + SS * [colsum01(W o z), R2(z), R3(z), R4(z)]
    backward T_m = W_m0*x0 + W_m1*x1 - x'_c(m)   (x'_c = scaled col sums)

  The FISTA iteration is run in a rescaled dual space (lam-hat = lam~ - b~,
  y-hat likewise; step folded into SS/b~) so one iteration is only:
    forward (8 DVE ops) + backward (9 DVE + 4 ACT ops)
    + arg=T+y (1) + lam' = max(arg,0)-b~ (fused STT, 1)
    + y' = (1+b)lam' - b lam  (fused LN_BWD_DX custom op, 1)

  The power iteration for L reuses the same forward/backward machinery with
  constant scale planes. All 330 iterations are fully unrolled (no loop
  back-edge cost); betas are host-precomputed fp32 constants.
"""
import os
from contextlib import ExitStack

import numpy as np

import concourse.bass as bass
import concourse.tile as tile
from concourse import mybir
from concourse.bass_utils import run_bass_kernel_spmd

f32 = mybir.dt.float32
AX = mybir.AxisListType
OP = mybir.AluOpType
AF = mybir.ActivationFunctionType

P = 128
F = 32
BPC = P * F            # samples per core
NCORES = 8
B_FULL = BPC * NCORES  # 32768

N_POWER = 30
N_FISTA = 300

MAX_OBS = 10
MAX_NEI = 7
BIG = 1000.0
PINV = np.array([0.5, 0.5, 1.0 / 200.0, 1.0 / 200.0, 1.0 / 200.0], np.float64)
K0 = float(np.float32(np.sqrt(PINV[0])))      # sqrt(1/2)
PINV2 = float(np.float32(PINV[2]))
SQ2 = float(np.float32(np.sqrt(2.0)))         # 2*K0 (= -q~ coefficient)

RAW_SPECS = [
    ("u_nominal", (BPC, 2)),
    ("v_current", (BPC, 1)),
    ("p_obs", (BPC, MAX_OBS, 2)),
    ("obs_mask", (BPC, MAX_OBS)),
    ("p_agents", (BPC, MAX_NEI, 2)),
    ("v_agents_local", (BPC, MAX_NEI, 2)),
    ("agents_mask", (BPC, MAX_NEI)),
    ("p_c_agent", (BPC, 1, 2)),
    ("v_c_agent", (BPC, 1, 2)),
    ("closest_mask", (BPC, 1)),
]
# All inputs are packed host-side into one (BPC, NFEAT) array: a single DMA
# means a single DMA-queue semaphore, keeping every instruction's sync-wait
# count below walrus's per-instruction limit.
NFEAT = 73
F_U, F_V, F_OBS, F_OM, F_AG, F_VA, F_AM, F_PC, F_VC, F_CM = 0, 2, 3, 23, 33, 47, 61, 68, 70, 72


def _betas(n):
    """Host fp32 replica of the on-device tk recursion."""
    one, half, four = np.float32(1.0), np.float32(0.5), np.float32(4.0)
    tk = np.float32(1.0)
    out = []
    for _ in range(n):
        tk1 = half * (one + np.sqrt(one + four * tk * tk, dtype=np.float32))
        beta = np.float32((tk - one) / tk1)
        out.append(float(beta))
        tk = tk1
    return out


# --------------------------------------------------------------------------
# emission helpers
# --------------------------------------------------------------------------
def _pl(t, i, n=1):
    """planes [i, i+n) of a plane-buffer tile as flat [P, n*F]."""
    return t[:, i * F:(i + n) * F]


def _pv(t, nplanes):
    """[P, nplanes, F] view of a plane-buffer tile."""
    return t[:].rearrange("p (m f) -> p m f", m=nplanes)


def _bc(plane_ap, n):
    """broadcast one [P, F] plane to [P, n, F] (step-0 middle dim)."""
    return plane_ap.unsqueeze(1).broadcast_to([P, n, F])


class Emit:
    def __init__(self, ctx, tc):
        self.tc = tc
        self.nc = tc.nc
        pool = ctx.enter_context(tc.tile_pool(name="state", bufs=1))
        self.fence_pool = ctx.enter_context(tc.tile_pool(name="fence", bufs=2))
        T = lambda n, tag: pool.tile([P, n * F], f32, name=tag, tag=tag)
        self.W = T(36, "W")          # Abar cols 0,1 in (row, col) pair order
        self.b = T(25, "b")          # unscaled h
        self.btil = T(25, "btil")    # sqrt(s) * b
        self.lamA = T(25, "lamA")
        self.lamB = T(25, "lamB")
        self.yh = T(25, "yh")        # y-hat (doubles as power-iteration v)
        self.Tbuf = T(25, "Tbuf")    # backward output (doubles as power w)
        self.prod = T(36, "prod")    # product scratch (also precompute scratch)
        self.ph = T(18, "ph")        # halves-add scratch for the fwd col sums
        self.X = T(5, "X")
        self.R = T(5, "R")
        self.SS = T(5, "SS")
        self.SSp = T(5, "SSp")
        self.Qadd = T(5, "Qadd")
        self.cs2 = T(2, "cs2")
        self.bx = T(2, "bx")
        self.u2 = T(2, "u2")
        self.sc1 = T(1, "sc1")       # small scalars-per-sample planes
        self.sc2 = T(1, "sc2")
        self.ns = T(1, "ns")
        self.rn = T(1, "rn")
        self.rs = T(1, "rs")
        self.sstar = T(1, "sstar")
        self.v2 = T(1, "v2")         # 2*v^2
        self.opack = T(2, "opack")
        self.tpA = T(25, "tpA")      # ACT-computed -beta*lam_prev (ping-pong)
        self.tpB = T(25, "tpB")

    def _act_fence(self, act_insts):
        """Tiny DVE memset carrying the sync-wait on ACT producers: walrus
        rejects instructions with >1 sync wait, and a DVE consumer of ACT
        output usually also needs its own-engine wait. The fence absorbs the
        ACT wait so the consumer keeps a single wait. Rotating 2-buf target
        keeps the fence's own WAW dep old enough to need no self-wait."""
        ft = self.fence_pool.tile([P, 1], f32, name="fence", tag="fence")
        ins = self.nc.vector.memset(ft[:], 0.0)
        for ai in act_insts:
            tile.add_dep_helper(ins.ins, ai.ins, sync=True, reason="act fence")
        return ins

    # ---------------- fwd/bwd machinery ----------------
    def forward(self, z, SSbuf, Qadd, X):
        nc = self.nc
        zv = _pv(z, 25)
        W4 = self.W[:].rearrange("p (r c f) -> p r c f", r=18, c=2)
        P4 = self.prod[:].rearrange("p (r c f) -> p r c f", r=18, c=2)
        # F1: per-sample products of cols 0,1 for the 18 geometric rows
        nc.vector.tensor_tensor(
            out=P4, in0=W4,
            in1=zv[:, 0:18].unsqueeze(2).broadcast_to([P, 18, 2, F]),
            op=OP.mult)
        # F2: column sums over the 18 rows -> cs2 = [S0raw, S1raw]. Two hops:
        # a contiguous halves-add folds 18 r-planes to 9, then the strided
        # reduce (1.66 ns/elem vs 1.04 contiguous) only reads half the data.
        nc.vector.tensor_tensor(
            out=self.ph[:], in0=self.prod[:, 0:9 * 2 * F],
            in1=self.prod[:, 9 * 2 * F:18 * 2 * F], op=OP.add)
        nc.vector.tensor_reduce(
            out=self.cs2[:],
            in_=self.ph[:].rearrange("p (r cf) -> p cf r", r=9),
            axis=AX.X, op=OP.add)
        # F3: box pair differences bx = [z22-z21, z24-z23]
        zbox = z[:, 21 * F:25 * F].rearrange("p (c g f) -> p c g f", c=2, g=2)
        nc.vector.tensor_tensor(
            out=self.bx[:].rearrange("p (c f) -> p c f", c=2),
            in0=zbox[:, :, 1, :], in1=zbox[:, :, 0, :], op=OP.subtract)
        # F4: R[0:2] = K0*bx + cs2
        nc.vector.scalar_tensor_tensor(
            out=_pl(self.R, 0, 2), in0=self.bx[:], scalar=K0, in1=self.cs2[:],
            op0=OP.mult, op1=OP.add)
        # F5/F6: R2 = sum(obs z)+z18 ; R3 = sum(nei z)+z19 ; R4 = z17+z20
        zf = z[:].rearrange("p (m f) -> p f m", m=25)
        nc.vector.tensor_reduce(out=_pl(self.R, 2), in_=zf[:, :, 0:10],
                                axis=AX.X, op=OP.add)
        nc.vector.tensor_tensor(out=_pl(self.R, 2), in0=_pl(self.R, 2),
                                in1=_pl(z, 18), op=OP.add)
        nc.vector.tensor_reduce(out=_pl(self.R, 3), in_=zf[:, :, 10:17],
                                axis=AX.X, op=OP.add)
        nc.vector.tensor_tensor(out=_pl(self.R, 3), in0=_pl(self.R, 3),
                                in1=_pl(z, 19), op=OP.add)
        nc.vector.tensor_tensor(out=_pl(self.R, 4), in0=_pl(z, 17),
                                in1=_pl(z, 20), op=OP.add)
        # F7/F8: X = SS*R (+ Qadd)
        nc.vector.tensor_tensor(out=X[:], in0=SSbuf[:], in1=self.R[:], op=OP.mult)
        if Qadd is not None:
            nc.vector.tensor_tensor(out=X[:], in0=X[:], in1=Qadd[:], op=OP.add)

    def backward_T(self, X):
        nc = self.nc
        Tb = self.Tbuf
        W4 = self.W[:].rearrange("p (r c f) -> p r c f", r=18, c=2)
        P4 = self.prod[:].rearrange("p (r c f) -> p r c f", r=18, c=2)
        x2 = X[:, 0:2 * F].rearrange("p (c f) -> p c f", c=2)
        # B1: products W[m,c] * x_c
        nc.vector.tensor_tensor(
            out=P4, in0=W4,
            in1=x2.unsqueeze(1).broadcast_to([P, 18, 2, F]), op=OP.mult)
        # B2: T[0:18] = pc0 + pc1
        nc.vector.tensor_tensor(
            out=_pl(Tb, 0, 18).rearrange("p (r f) -> p r f", r=18),
            in0=P4[:, :, 0, :], in1=P4[:, :, 1, :], op=OP.add)
        # B3-B5: subtract shared scaled-colsum planes
        nc.vector.tensor_tensor(
            out=_pl(Tb, 0, 10).rearrange("p (r f) -> p r f", r=10),
            in0=_pl(Tb, 0, 10).rearrange("p (r f) -> p r f", r=10),
            in1=_bc(_pl(self.X, 2), 10), op=OP.subtract)
        nc.vector.tensor_tensor(
            out=_pl(Tb, 10, 7).rearrange("p (r f) -> p r f", r=7),
            in0=_pl(Tb, 10, 7).rearrange("p (r f) -> p r f", r=7),
            in1=_bc(_pl(self.X, 3), 7), op=OP.subtract)
        nc.vector.tensor_tensor(out=_pl(Tb, 17), in0=_pl(Tb, 17),
                                in1=_pl(self.X, 4), op=OP.subtract)
        # B6 (ACT): T[slack] = -x'[2:5]
        a1 = nc.scalar.activation(_pl(Tb, 18, 3), _pl(self.X, 2, 3), AF.Copy, scale=-1.0)
        # B7-B9 (ACT): box rows +-K0*x0, +-K0*x1
        a2 = nc.scalar.activation(self.u2[:], _pl(self.X, 0, 2), AF.Copy, scale=-K0)
        tbox = Tb[:, 21 * F:25 * F].rearrange("p (c g f) -> p c g f", c=2, g=2)
        u2v = self.u2[:].rearrange("p (c f) -> p c f", c=2)
        a3 = nc.scalar.activation(tbox[:, :, 0, :], u2v, AF.Copy)
        a4 = nc.scalar.activation(tbox[:, :, 1, :], u2v, AF.Copy, scale=-1.0)
        self._act_fence([a1, a2, a3, a4])

    # ---------------- precompute ----------------
    def precompute(self, natt):
        nc = self.nc
        Wv = _pv(self.W, 36)
        bv = _pv(self.b, 25)
        STT = nc.vector.scalar_tensor_tensor
        TT = nc.vector.tensor_tensor

        # feature access patterns into the packed native tile
        pk = natt[:].rearrange("p (f a) -> p a f", a=NFEAT)
        self.pk = pk
        obs = pk[:, F_OBS:F_OBS + 20, :].rearrange("p (o c) f -> p o c f", c=2)
        lx, ly = obs[:, :, 0, :], obs[:, :, 1, :]
        om = pk[:, F_OM:F_OM + MAX_OBS, :]
        ag = pk[:, F_AG:F_AG + 14, :].rearrange("p (o c) f -> p o c f", c=2)
        ax, ay = ag[:, :, 0, :], ag[:, :, 1, :]
        va = pk[:, F_VA:F_VA + 14, :].rearrange("p (o c) f -> p o c f", c=2)
        vjx, vjy = va[:, :, 0, :], va[:, :, 1, :]
        am = pk[:, F_AM:F_AM + MAX_NEI, :]
        cx, cy = pk[:, F_PC, :], pk[:, F_PC + 1, :]
        cvx, cvy = pk[:, F_VC, :], pk[:, F_VC + 1, :]
        cm = pk[:, F_CM, :]
        v = pk[:, F_V, :]

        # v2 = 2*v^2
        STT(out=self.v2[:], in0=v, scalar=2.0, in1=v, op0=OP.mult, op1=OP.mult)

        sA = self.prod[:, 0:10 * F].rearrange("p (o f) -> p o f", o=10)
        sB = self.prod[:, 10 * F:20 * F].rearrange("p (o f) -> p o f", o=10)

        # ---- obs rows (planes 0-9; W pairs 0..19) ----
        W0 = Wv[:, 0:20].rearrange("p (o c) f -> p o c f", c=2)[:, :, 0, :]
        W1 = Wv[:, 0:20].rearrange("p (o c) f -> p o c f", c=2)[:, :, 1, :]
        STT(out=W0, in0=lx, scalar=2.0 * K0, in1=om, op0=OP.mult, op1=OP.mult)
        STT(out=sA, in0=ly, scalar=2.0 * K0, in1=_bc(v, 10), op0=OP.mult, op1=OP.mult)
        TT(out=W1, in0=sA, in1=om, op=OP.mult)
        # h_rhs = 2v^2 - 6 lx v + 2 lx^2 + 2 ly^2 - 0.5
        STT(out=sA, in0=lx, scalar=-6.0, in1=_bc(v, 10), op0=OP.mult, op1=OP.mult)
        STT(out=sB, in0=lx, scalar=2.0, in1=lx, op0=OP.mult, op1=OP.mult)
        TT(out=sA, in0=sA, in1=sB, op=OP.add)
        STT(out=sB, in0=ly, scalar=2.0, in1=ly, op0=OP.mult, op1=OP.mult)
        TT(out=sA, in0=sA, in1=sB, op=OP.add)
        TT(out=sA, in0=sA, in1=_bc(self.v2[:], 10), op=OP.add)
        nc.vector.tensor_scalar(out=sA, in0=sA, scalar1=-0.5, scalar2=None, op0=OP.add)
        # mask: b = (h - BIG)*m + BIG
        STT(out=sA, in0=sA, scalar=-BIG, in1=om, op0=OP.add, op1=OP.mult)
        nc.vector.tensor_scalar(out=bv[:, 0:10], in0=sA, scalar1=BIG, scalar2=None, op0=OP.add)

        # ---- nei rows (planes 10-16; W pairs 20..33) ----
        sA7 = self.prod[:, 0:7 * F].rearrange("p (o f) -> p o f", o=7)
        sB7 = self.prod[:, 7 * F:14 * F].rearrange("p (o f) -> p o f", o=7)
        sC7 = self.prod[:, 14 * F:21 * F].rearrange("p (o f) -> p o f", o=7)
        W0 = Wv[:, 20:34].rearrange("p (o c) f -> p o c f", c=2)[:, :, 0, :]
        W1 = Wv[:, 20:34].rearrange("p (o c) f -> p o c f", c=2)[:, :, 1, :]
        STT(out=W0, in0=ax, scalar=2.0 * K0, in1=am, op0=OP.mult, op1=OP.mult)
        STT(out=sA7, in0=ay, scalar=2.0 * K0, in1=_bc(v, 7), op0=OP.mult, op1=OP.mult)
        STT(out=sB7, in0=ay, scalar=-2.0 * K0, in1=vjx, op0=OP.mult, op1=OP.mult)
        TT(out=sA7, in0=sA7, in1=sB7, op=OP.add)
        STT(out=sB7, in0=ax, scalar=2.0 * K0, in1=vjy, op0=OP.mult, op1=OP.mult)
        TT(out=sA7, in0=sA7, in1=sB7, op=OP.add)
        TT(out=W1, in0=sA7, in1=am, op=OP.mult)
        # h = 2v^2 - 4 v vjx + 2 vjx^2 + 2 vjy^2 - 6 ax v + 6 ax vjx + 6 ay vjy
        #     + 2 ax^2 + 2 ay^2 - 1.28
        STT(out=sA7, in0=vjx, scalar=-4.0, in1=_bc(v, 7), op0=OP.mult, op1=OP.mult)
        STT(out=sB7, in0=vjx, scalar=2.0, in1=vjx, op0=OP.mult, op1=OP.mult)
        TT(out=sA7, in0=sA7, in1=sB7, op=OP.add)
        STT(out=sB7, in0=vjy, scalar=2.0, in1=vjy, op0=OP.mult, op1=OP.mult)
        TT(out=sA7, in0=sA7, in1=sB7, op=OP.add)
        STT(out=sB7, in0=ax, scalar=-6.0, in1=_bc(v, 7), op0=OP.mult, op1=OP.mult)
        TT(out=sA7, in0=sA7, in1=sB7, op=OP.add)
        STT(out=sB7, in0=ax, scalar=6.0, in1=vjx, op0=OP.mult, op1=OP.mult)
        TT(out=sA7, in0=sA7, in1=sB7, op=OP.add)
        STT(out=sB7, in0=ay, scalar=6.0, in1=vjy, op0=OP.mult, op1=OP.mult)
        TT(out=sA7, in0=sA7, in1=sB7, op=OP.add)
        STT(out=sB7, in0=ax, scalar=2.0, in1=ax, op0=OP.mult, op1=OP.mult)
        TT(out=sA7, in0=sA7, in1=sB7, op=OP.add)
        STT(out=sB7, in0=ay, scalar=2.0, in1=ay, op0=OP.mult, op1=OP.mult)
        TT(out=sA7, in0=sA7, in1=sB7, op=OP.add)
        TT(out=sA7, in0=sA7, in1=_bc(self.v2[:], 7), op=OP.add)
        nc.vector.tensor_scalar(out=sA7, in0=sA7, scalar1=-1.28, scalar2=None, op0=OP.add)
        STT(out=sA7, in0=sA7, scalar=-BIG, in1=am, op0=OP.add, op1=OP.mult)
        nc.vector.tensor_scalar(out=bv[:, 10:17], in0=sA7, scalar1=BIG, scalar2=None, op0=OP.add)

        # ---- cn row (plane 17; W pair 34,35) ----
        s1 = _pl(self.prod, 0)
        s2 = _pl(self.prod, 1)
        STT(out=Wv[:, 34], in0=cx, scalar=-2.0 * K0, in1=cm, op0=OP.mult, op1=OP.mult)
        STT(out=s1, in0=cy, scalar=-2.0 * K0, in1=v, op0=OP.mult, op1=OP.mult)
        STT(out=s2, in0=cy, scalar=2.0 * K0, in1=cvx, op0=OP.mult, op1=OP.mult)
        TT(out=s1, in0=s1, in1=s2, op=OP.add)
        STT(out=s2, in0=cx, scalar=-2.0 * K0, in1=cvy, op0=OP.mult, op1=OP.mult)
        TT(out=s1, in0=s1, in1=s2, op=OP.add)
        TT(out=Wv[:, 35], in0=s1, in1=cm, op=OP.mult)
        # h = -2v^2 + 4 v cvx - 2 cvx^2 - 2 cvy^2 + 6 cx v - 6 cx cvx - 6 cy cvy
        #     - 2 cx^2 - 2 cy^2 + 50
        STT(out=s1, in0=cvx, scalar=4.0, in1=v, op0=OP.mult, op1=OP.mult)
        STT(out=s2, in0=cvx, scalar=-2.0, in1=cvx, op0=OP.mult, op1=OP.mult)
        TT(out=s1, in0=s1, in1=s2, op=OP.add)
        STT(out=s2, in0=cvy, scalar=-2.0, in1=cvy, op0=OP.mult, op1=OP.mult)
        TT(out=s1, in0=s1, in1=s2, op=OP.add)
        STT(out=s2, in0=cx, scalar=6.0, in1=v, op0=OP.mult, op1=OP.mult)
        TT(out=s1, in0=s1, in1=s2, op=OP.add)
        STT(out=s2, in0=cx, scalar=-6.0, in1=cvx, op0=OP.mult, op1=OP.mult)
        TT(out=s1, in0=s1, in1=s2, op=OP.add)
        STT(out=s2, in0=cy, scalar=-6.0, in1=cvy, op0=OP.mult, op1=OP.mult)
        TT(out=s1, in0=s1, in1=s2, op=OP.add)
        STT(out=s2, in0=cx, scalar=-2.0, in1=cx, op0=OP.mult, op1=OP.mult)
        TT(out=s1, in0=s1, in1=s2, op=OP.add)
        STT(out=s2, in0=cy, scalar=-2.0, in1=cy, op0=OP.mult, op1=OP.mult)
        TT(out=s1, in0=s1, in1=s2, op=OP.add)
        TT(out=s1, in0=s1, in1=self.v2[:], op=OP.subtract)
        nc.vector.tensor_scalar(out=s1, in0=s1, scalar1=50.0, scalar2=None, op0=OP.add)
        STT(out=s1, in0=s1, scalar=-BIG, in1=cm, op0=OP.add, op1=OP.mult)
        nc.vector.tensor_scalar(out=_pl(self.b, 17), in0=s1, scalar1=BIG, scalar2=None, op0=OP.add)

        # ---- slack/box b, power scale planes ----
        nc.vector.memset(_pl(self.b, 18, 3), 0.0)
        nc.vector.memset(_pl(self.b, 21, 2), 2.0)
        nc.vector.memset(_pl(self.b, 23, 2), 1.0)
        nc.vector.memset(_pl(self.SSp, 0, 2), 1.0)
        for c in range(3):
            nc.vector.memset(_pl(self.SSp, 2 + c), -PINV2)


    def emit_rsqrt(self, dst, src, newton=0):
        """dst = rsqrt(src) via reciprocal + ACT Sqrt seed + Newton steps."""
        nc = self.nc
        nc.vector.reciprocal(out=self.sc1[:], in_=src)
        a = nc.scalar.activation(dst, self.sc1[:], AF.Sqrt)
        self._act_fence([a])
        for _ in range(newton):
            nc.vector.tensor_tensor(out=self.sc1[:], in0=dst, in1=dst, op=OP.mult)
            nc.vector.tensor_tensor(out=self.sc1[:], in0=src, in1=self.sc1[:], op=OP.mult)
            nc.vector.tensor_scalar(out=self.sc1[:], in0=self.sc1[:],
                                    scalar1=-0.5, scalar2=1.5, op0=OP.mult, op1=OP.add)
            nc.vector.tensor_tensor(out=dst, in0=dst, in1=self.sc1[:], op=OP.mult)

    # ---------------- power iteration + step ----------------
    def power_phase(self, n_power):
        nc = self.nc
        TT = nc.vector.tensor_tensor
        nc.vector.memset(self.yh[:], 1.0)
        sq = self.prod[:, 0:25 * F]
        for it in range(n_power):
            self.forward(self.yh, self.SSp, None, self.X)
            self.backward_T(self.X)
            # normalize: yh = w * rsqrt(sum w^2). Intermediate normalizations
            # only bound the range (direction is scale-invariant); the last one
            # enters the Rayleigh quotient, so refine it.
            TT(out=sq, in0=self.Tbuf[:], in1=self.Tbuf[:], op=OP.mult)
            nc.vector.tensor_reduce(
                out=self.ns[:], in_=sq.rearrange("p (m f) -> p f m", m=25),
                axis=AX.X, op=OP.add)
            self.emit_rsqrt(self.rn[:], self.ns[:],
                            newton=2 if it == n_power - 1 else 0)
            TT(out=_pv(self.yh, 25), in0=_pv(self.Tbuf, 25),
               in1=_bc(self.rn[:], 25), op=OP.mult)
        # Rayleigh L = v . (M v); then rs = rsqrt(L + 1e-6), s* = rs^2
        self.forward(self.yh, self.SSp, None, self.X)
        self.backward_T(self.X)
        TT(out=sq, in0=self.yh[:], in1=self.Tbuf[:], op=OP.mult)
        nc.vector.tensor_reduce(
            out=self.ns[:], in_=sq.rearrange("p (m f) -> p f m", m=25),
            axis=AX.X, op=OP.add)
        nc.vector.tensor_scalar(out=self.ns[:], in0=self.ns[:],
                                scalar1=1e-6, scalar2=None, op0=OP.add)
        self.emit_rsqrt(self.rs[:], self.ns[:], newton=2)
        TT(out=self.sstar[:], in0=self.rs[:], in1=self.rs[:], op=OP.mult)

    # ---------------- FISTA setup ----------------
    def fista_setup(self):
        nc = self.nc
        TT = nc.vector.tensor_tensor
        # btil = b * rs
        TT(out=_pv(self.btil, 25), in0=_pv(self.b, 25), in1=_bc(self.rs[:], 25),
           op=OP.mult)
        # SS = [-s*, -s*, PINV2*s* x3]
        a1 = nc.scalar.activation(
            _pl(self.SS, 0, 2).rearrange("p (c f) -> p c f", c=2),
            _bc(self.sstar[:], 2), AF.Copy, scale=-1.0)
        a2 = nc.scalar.activation(
            _pl(self.SS, 2, 3).rearrange("p (c f) -> p c f", c=3),
            _bc(self.sstar[:], 3), AF.Copy, scale=PINV2)
        self._act_fence([a1, a2])
        # Qadd = SS*FWD(btil) + rs * q~   (q~ = [sqrt2 u0, sqrt2 u1, 0,0,0])
        self.forward(self.btil, self.SS, None, self.Qadd)
        uap = self.pk[:, F_U:F_U + 2, :]
        nc.vector.scalar_tensor_tensor(
            out=self.u2[:].rearrange("p (c f) -> p c f", c=2),
            in0=uap, scalar=SQ2, in1=_bc(self.rs[:], 2), op0=OP.mult, op1=OP.mult)
        TT(out=_pl(self.Qadd, 0, 2), in0=_pl(self.Qadd, 0, 2), in1=self.u2[:],
           op=OP.add)
        # lam = yh = -btil
        a1 = nc.scalar.activation(self.lamA[:], self.btil[:], AF.Copy, scale=-1.0)
        a2 = nc.scalar.activation(self.yh[:], self.btil[:], AF.Copy, scale=-1.0)
        self._act_fence([a1, a2])

    # ---------------- FISTA loop ----------------
    def fista(self, n_fista):
        nc = self.nc
        betas = _betas(n_fista)
        lams = [self.lamA, self.lamB]
        # tp = -beta*lam_prev runs on the (otherwise idle) ACT engine. lam_prev
        # is ready at iteration start, so the ACT op overlaps the whole DVE
        # chain; the DVE only pays one fused STT for the momentum update:
        #   yh = (1+beta)*lam_new + tp
        # Rounding differs from (lam_new - lam_prev)*beta + lam_new by ~1 ulp,
        # which the harness tolerance absorbs.
        tps = [self.tpA, self.tpB]
        for it in range(n_fista):
            lam_prev = lams[it % 2]
            lam_new = lams[(it + 1) % 2]
            beta = betas[it]
            tp = tps[it % 2]
            a_tp = nc.scalar.activation(tp[:], lam_prev[:], AF.Copy,
                                        scale=-float(beta))
            self.forward(self.yh, self.SS, self.Qadd, self.X)
            self.backward_T(self.X)
            # arg = T + yh ; lam_new = max(arg,0) - btil
            nc.vector.tensor_tensor(out=self.Tbuf[:], in0=self.Tbuf[:],
                                    in1=self.yh[:], op=OP.add)
            nc.vector.scalar_tensor_tensor(
                out=lam_new[:], in0=self.Tbuf[:], scalar=0.0, in1=self.btil[:],
                op0=OP.max, op1=OP.subtract)
            # yh = (1+beta)*lam_new + tp
            self._act_fence([a_tp])
            nc.vector.scalar_tensor_tensor(
                out=self.yh[:], in0=lam_new[:], scalar=float(1.0 + beta),
                in1=tp[:], op0=OP.mult, op1=OP.add)
        return lams[n_fista % 2]

    # ---------------- finale ----------------
    def finale(self, lam_final, out_dram):
        nc = self.nc
        TT = nc.vector.tensor_tensor
        self.forward(lam_final, self.SS, self.Qadd, self.X)
        # u = K0 * X[0:2] / rs ; 1/rs = sqrt(L+1e-6) = ns * rs
        TT(out=self.sc1[:], in0=self.ns[:], in1=self.rs[:], op=OP.mult)
        a = nc.scalar.activation(self.sc2[:], self.sc1[:], AF.Copy, scale=K0)
        self.last_act = a
        self._act_fence([a])
        self.last_dve = TT(out=self.opack[:].rearrange("p (f c) -> p c f", c=2),
           in0=_pl(self.X, 0, 2).rearrange("p (c f) -> p c f", c=2),
           in1=_bc(self.sc2[:], 2), op=OP.mult)
        self.out_dma = nc.sync.dma_start(
            out=out_dram.ap().rearrange("(p f) c -> p (f c)", p=P),
            in_=self.opack[:])

    def terminals(self):
        return [self.in_dma, self.last_act, self.last_dve, self.out_dma]


def build_nc(n_power=N_POWER, n_fista=N_FISTA):
    nc = bass.Bass("TRN2")
    din = nc.dram_tensor("packed", [BPC, NFEAT], f32, kind="ExternalInput")
    dout = nc.dram_tensor("u_safe", [BPC, 2], f32, kind="ExternalOutput")

    with tile.TileContext(nc) as tc:
        with ExitStack() as ctx:
            em = Emit(ctx, tc)
            natpool = ctx.enter_context(tc.tile_pool(name="nat", bufs=1))
            natt = natpool.tile([P, F * NFEAT], f32, name="nat", tag="nat")
            # single SWDGE DMA -> one DMA semaphore for all downstream waits
            em.in_dma = nc.gpsimd.dma_start(
                out=natt[:], in_=din.ap().rearrange("(p f) a -> p (f a)", p=P))
            em.precompute(natt)
            em.power_phase(n_power)
            em.fista_setup()
            lam_final = em.fista(n_fista)
            em.finale(lam_final, dout)
            # Exit fence: the tile-exit drain would wait on every active proc
            # (ACT, DVE, DMA queues) at once, exceeding walrus's one-sync-wait
            # -per-instruction limit. Chain sync-engine NOPs, one dep each, so
            # the SP engine observes every proc before the drain.
            for ti in em.terminals():
                nop = nc.sync.nop()
                tile.add_dep_helper(nop.ins, ti.ins, sync=True,
                                    reason="exit fence")
    return nc


_NC_CACHE = {}


def _get_nc(n_power=N_POWER, n_fista=N_FISTA):
    key = (n_power, n_fista)
    if key not in _NC_CACHE:
        _NC_CACHE[key] = build_nc(n_power, n_fista)
    return _NC_CACHE[key]


def pack_inputs(inputs, lo, hi):
    """Pack the raw input dict (rows [lo, hi)) into one (n, NFEAT) array."""
    n = hi - lo
    cols = [np.asarray(inputs[name], np.float32)[lo:hi].reshape(n, -1)
            for name, _ in RAW_SPECS]
    return np.ascontiguousarray(np.concatenate(cols, axis=1))


def kernel(**inputs):
    """Full-input entry point: shard batch over 8 cores, run, gather."""
    nc = _get_nc()
    in_maps = [{"packed": pack_inputs(inputs, c * BPC, (c + 1) * BPC)}
               for c in range(NCORES)]
    res = run_bass_kernel_spmd(nc, in_maps, list(range(NCORES)))
    return np.concatenate([res.results[c]["u_safe"] for c in range(NCORES)],
                          axis=0)


if __name__ == "__main__":
    # smoke test on random data against a tiny numpy reference path
    rng = np.random.default_rng(0)
    demo = {
        "u_nominal": rng.standard_normal((B_FULL, 2)).astype(np.float32),
        "v_current": rng.uniform(0, 1, (B_FULL, 1)).astype(np.float32),
        "p_obs": (2 * rng.standard_normal((B_FULL, MAX_OBS, 2))).astype(np.float32),
        "obs_mask": np.ones((B_FULL, MAX_OBS), np.float32),
        "p_agents": (2 * rng.standard_normal((B_FULL, MAX_NEI, 2))).astype(np.float32),
        "v_agents_local": rng.standard_normal((B_FULL, MAX_NEI, 2)).astype(np.float32),
        "agents_mask": np.ones((B_FULL, MAX_NEI), np.float32),
        "p_c_agent": (2 * rng.standard_normal((B_FULL, 1, 2))).astype(np.float32),
        "v_c_agent": rng.standard_normal((B_FULL, 1, 2)).astype(np.float32),
        "closest_mask": np.ones((B_FULL, 1), np.float32),
    }
    out = kernel(**demo)
    print(out.shape, out.dtype, np.abs(out).max())



# revision 6
# speedup vs baseline: 1.1330x; 1.0252x over previous
"""Trainium2 Bass kernel for the DifferentiableCBFLayer batched dual-FISTA QP.

Strategy (pure data parallel, 8 cores x 4096 samples):
  Each core holds its 4096 samples as [128 partitions, 32 free] f32 "planes"
  (one plane per scalar quantity). The 26-row constraint system is reduced to
  25 rows (the all-zero "extra" row provably contributes nothing) with row
  order [obs x10, nei x7, cn, slack x3, box x4]. Only the 36 entries of
  columns 0,1 for the 18 geometric rows are per-sample; columns 2-4 are
  per-row constants (+- sqrt(Pinv_c) * mask), which lets both matvec
  directions run as a handful of large multi-plane DVE instructions:

    forward  x~ = Qadd + SS * [colsum01(W o z), R2(z), R3(z), R4(z)]
    backward T_m = W_m0*x0 + W_m1*x1 - x'_c(m)   (x'_c = scaled col sums)

  The FISTA iteration is run in a rescaled dual space (lam-hat = lam~ - b~,
  y-hat likewise; step folded into SS/b~) so one iteration is only:
    forward (8 DVE ops) + backward (9 DVE + 4 ACT ops)
    + arg=T+y (1) + lam' = max(arg,0)-b~ (fused STT, 1)
    + y' = (1+b)lam' - b lam  (fused LN_BWD_DX custom op, 1)

  The power iteration for L reuses the same forward/backward machinery with
  constant scale planes. All 330 iterations are fully unrolled (no loop
  back-edge cost); betas are host-precomputed fp32 constants.
"""
import os
from contextlib import ExitStack

import numpy as np

import concourse.bass as bass
import concourse.tile as tile
from concourse import mybir
from concourse.bass_utils import run_bass_kernel_spmd

f32 = mybir.dt.float32
AX = mybir.AxisListType
OP = mybir.AluOpType
AF = mybir.ActivationFunctionType

P = 128
F = 32
BPC = P * F            # samples per core
NCORES = 8
B_FULL = BPC * NCORES  # 32768

N_POWER = 30
N_FISTA = 300

MAX_OBS = 10
MAX_NEI = 7
BIG = 1000.0
PINV = np.array([0.5, 0.5, 1.0 / 200.0, 1.0 / 200.0, 1.0 / 200.0], np.float64)
K0 = float(np.float32(np.sqrt(PINV[0])))      # sqrt(1/2)
PINV2 = float(np.float32(PINV[2]))
SQ2 = float(np.float32(np.sqrt(2.0)))         # 2*K0 (= -q~ coefficient)

RAW_SPECS = [
    ("u_nominal", (BPC, 2)),
    ("v_current", (BPC, 1)),
    ("p_obs", (BPC, MAX_OBS, 2)),
    ("obs_mask", (BPC, MAX_OBS)),
    ("p_agents", (BPC, MAX_NEI, 2)),
    ("v_agents_local", (BPC, MAX_NEI, 2)),
    ("agents_mask", (BPC, MAX_NEI)),
    ("p_c_agent", (BPC, 1, 2)),
    ("v_c_agent", (BPC, 1, 2)),
    ("closest_mask", (BPC, 1)),
]
# All inputs are packed host-side into one (BPC, NFEAT) array: a single DMA
# means a single DMA-queue semaphore, keeping every instruction's sync-wait
# count below walrus's per-instruction limit.
NFEAT = 73
F_U, F_V, F_OBS, F_OM, F_AG, F_VA, F_AM, F_PC, F_VC, F_CM = 0, 2, 3, 23, 33, 47, 61, 68, 70, 72


def _betas(n):
    """Host fp32 replica of the on-device tk recursion."""
    one, half, four = np.float32(1.0), np.float32(0.5), np.float32(4.0)
    tk = np.float32(1.0)
    out = []
    for _ in range(n):
        tk1 = half * (one + np.sqrt(one + four * tk * tk, dtype=np.float32))
        beta = np.float32((tk - one) / tk1)
        out.append(float(beta))
        tk = tk1
    return out


# --------------------------------------------------------------------------
# emission helpers
# --------------------------------------------------------------------------
def _pl(t, i, n=1):
    """planes [i, i+n) of a plane-buffer tile as flat [P, n*F]."""
    return t[:, i * F:(i + n) * F]


def _pv(t, nplanes):
    """[P, nplanes, F] view of a plane-buffer tile."""
    return t[:].rearrange("p (m f) -> p m f", m=nplanes)


def _bc(plane_ap, n):
    """broadcast one [P, F] plane to [P, n, F] (step-0 middle dim)."""
    return plane_ap.unsqueeze(1).broadcast_to([P, n, F])


class Emit:
    def __init__(self, ctx, tc):
        self.tc = tc
        self.nc = tc.nc
        pool = ctx.enter_context(tc.tile_pool(name="state", bufs=1))
        self.fence_pool = ctx.enter_context(tc.tile_pool(name="fence", bufs=2))
        T = lambda n, tag: pool.tile([P, n * F], f32, name=tag, tag=tag)
        self.W = T(36, "W")          # Abar cols 0,1 in (row, col) pair order
        self.b = T(25, "b")          # unscaled h
        self.btil = T(25, "btil")    # sqrt(s) * b
        self.lamA = T(25, "lamA")
        self.lamB = T(25, "lamB")
        self.yh = T(25, "yh")        # y-hat (doubles as power-iteration v)
        self.Tbuf = T(25, "Tbuf")    # backward output (doubles as power w)
        self.prod = T(36, "prod")    # product scratch (also precompute scratch)
        self.ph = T(18, "ph")        # halves-add scratch for the fwd col sums
        self.X = T(5, "X")
        self.R = T(5, "R")
        self.SS = T(5, "SS")
        self.SSp = T(5, "SSp")
        self.Qadd = T(5, "Qadd")
        self.cs2 = T(2, "cs2")
        self.bx = T(2, "bx")
        self.u2 = T(2, "u2")
        self.sc1 = T(1, "sc1")       # small scalars-per-sample planes
        self.sc2 = T(1, "sc2")
        self.ns = T(1, "ns")
        self.rn = T(1, "rn")
        self.rs = T(1, "rs")
        self.sstar = T(1, "sstar")
        self.v2 = T(1, "v2")         # 2*v^2
        self.opack = T(2, "opack")
        self.tpA = T(25, "tpA")      # ACT-computed -beta*lam_prev (ping-pong)
        self.tpB = T(25, "tpB")

    def _act_fence(self, act_insts):
        """Tiny DVE memset carrying the sync-wait on ACT producers: walrus
        rejects instructions with >1 sync wait, and a DVE consumer of ACT
        output usually also needs its own-engine wait. The fence absorbs the
        ACT wait so the consumer keeps a single wait. Rotating 2-buf target
        keeps the fence's own WAW dep old enough to need no self-wait."""
        ft = self.fence_pool.tile([P, 1], f32, name="fence", tag="fence")
        ins = self.nc.vector.memset(ft[:], 0.0)
        for ai in act_insts:
            tile.add_dep_helper(ins.ins, ai.ins, sync=True, reason="act fence")
        return ins

    # ---------------- fwd/bwd machinery ----------------
    def forward(self, z, SSbuf, Qadd, X):
        nc = self.nc
        zv = _pv(z, 25)
        W4 = self.W[:].rearrange("p (r c f) -> p r c f", r=18, c=2)
        P4 = self.prod[:].rearrange("p (r c f) -> p r c f", r=18, c=2)
        # F1: per-sample products of cols 0,1 for the 18 geometric rows
        nc.vector.tensor_tensor(
            out=P4, in0=W4,
            in1=zv[:, 0:18].unsqueeze(2).broadcast_to([P, 18, 2, F]),
            op=OP.mult)
        # F2: column sums over the 18 rows -> cs2 = [S0raw, S1raw]. Two hops:
        # a contiguous halves-add folds 18 r-planes to 9, then the strided
        # reduce (1.66 ns/elem vs 1.04 contiguous) only reads half the data.
        nc.vector.tensor_tensor(
            out=self.ph[:], in0=self.prod[:, 0:9 * 2 * F],
            in1=self.prod[:, 9 * 2 * F:18 * 2 * F], op=OP.add)
        nc.vector.tensor_reduce(
            out=self.cs2[:],
            in_=self.ph[:].rearrange("p (r cf) -> p cf r", r=9),
            axis=AX.X, op=OP.add)
        # F3: box pair differences bx = [z22-z21, z24-z23]
        zbox = z[:, 21 * F:25 * F].rearrange("p (c g f) -> p c g f", c=2, g=2)
        nc.vector.tensor_tensor(
            out=self.bx[:].rearrange("p (c f) -> p c f", c=2),
            in0=zbox[:, :, 1, :], in1=zbox[:, :, 0, :], op=OP.subtract)
        # F4: R[0:2] = K0*bx + cs2
        nc.vector.scalar_tensor_tensor(
            out=_pl(self.R, 0, 2), in0=self.bx[:], scalar=K0, in1=self.cs2[:],
            op0=OP.mult, op1=OP.add)
        # F5/F6: R2 = sum(obs z)+z18 ; R3 = sum(nei z)+z19 ; R4 = z17+z20
        zf = z[:].rearrange("p (m f) -> p f m", m=25)
        nc.vector.tensor_reduce(out=_pl(self.R, 2), in_=zf[:, :, 0:10],
                                axis=AX.X, op=OP.add)
        nc.vector.tensor_tensor(out=_pl(self.R, 2), in0=_pl(self.R, 2),
                                in1=_pl(z, 18), op=OP.add)
        nc.vector.tensor_reduce(out=_pl(self.R, 3), in_=zf[:, :, 10:17],
                                axis=AX.X, op=OP.add)
        nc.vector.tensor_tensor(out=_pl(self.R, 3), in0=_pl(self.R, 3),
                                in1=_pl(z, 19), op=OP.add)
        nc.vector.tensor_tensor(out=_pl(self.R, 4), in0=_pl(z, 17),
                                in1=_pl(z, 20), op=OP.add)
        # F7/F8: X = SS*R (+ Qadd)
        nc.vector.tensor_tensor(out=X[:], in0=SSbuf[:], in1=self.R[:], op=OP.mult)
        if Qadd is not None:
            nc.vector.tensor_tensor(out=X[:], in0=X[:], in1=Qadd[:], op=OP.add)

    def backward_T(self, X):
        nc = self.nc
        Tb = self.Tbuf
        W4 = self.W[:].rearrange("p (r c f) -> p r c f", r=18, c=2)
        P4 = self.prod[:].rearrange("p (r c f) -> p r c f", r=18, c=2)
        x2 = X[:, 0:2 * F].rearrange("p (c f) -> p c f", c=2)
        # B1: products W[m,c] * x_c
        nc.vector.tensor_tensor(
            out=P4, in0=W4,
            in1=x2.unsqueeze(1).broadcast_to([P, 18, 2, F]), op=OP.mult)
        # B2: T[0:18] = pc0 + pc1
        nc.vector.tensor_tensor(
            out=_pl(Tb, 0, 18).rearrange("p (r f) -> p r f", r=18),
            in0=P4[:, :, 0, :], in1=P4[:, :, 1, :], op=OP.add)
        # B3-B5: subtract shared scaled-colsum planes
        nc.vector.tensor_tensor(
            out=_pl(Tb, 0, 10).rearrange("p (r f) -> p r f", r=10),
            in0=_pl(Tb, 0, 10).rearrange("p (r f) -> p r f", r=10),
            in1=_bc(_pl(self.X, 2), 10), op=OP.subtract)
        nc.vector.tensor_tensor(
            out=_pl(Tb, 10, 7).rearrange("p (r f) -> p r f", r=7),
            in0=_pl(Tb, 10, 7).rearrange("p (r f) -> p r f", r=7),
            in1=_bc(_pl(self.X, 3), 7), op=OP.subtract)
        nc.vector.tensor_tensor(out=_pl(Tb, 17), in0=_pl(Tb, 17),
                                in1=_pl(self.X, 4), op=OP.subtract)
        # B6 (ACT): T[slack] = -x'[2:5]
        a1 = nc.scalar.activation(_pl(Tb, 18, 3), _pl(self.X, 2, 3), AF.Copy, scale=-1.0)
        # B7-B9 (ACT): box rows +-K0*x0, +-K0*x1
        a2 = nc.scalar.activation(self.u2[:], _pl(self.X, 0, 2), AF.Copy, scale=-K0)
        tbox = Tb[:, 21 * F:25 * F].rearrange("p (c g f) -> p c g f", c=2, g=2)
        u2v = self.u2[:].rearrange("p (c f) -> p c f", c=2)
        a3 = nc.scalar.activation(tbox[:, :, 0, :], u2v, AF.Copy)
        a4 = nc.scalar.activation(tbox[:, :, 1, :], u2v, AF.Copy, scale=-1.0)
        self._act_fence([a1, a2, a3, a4])

    # ---------------- precompute ----------------
    def precompute(self, natt):
        nc = self.nc
        Wv = _pv(self.W, 36)
        bv = _pv(self.b, 25)
        STT = nc.vector.scalar_tensor_tensor
        TT = nc.vector.tensor_tensor

        # feature access patterns into the packed native tile
        pk = natt[:].rearrange("p (f a) -> p a f", a=NFEAT)
        self.pk = pk
        obs = pk[:, F_OBS:F_OBS + 20, :].rearrange("p (o c) f -> p o c f", c=2)
        lx, ly = obs[:, :, 0, :], obs[:, :, 1, :]
        om = pk[:, F_OM:F_OM + MAX_OBS, :]
        ag = pk[:, F_AG:F_AG + 14, :].rearrange("p (o c) f -> p o c f", c=2)
        ax, ay = ag[:, :, 0, :], ag[:, :, 1, :]
        va = pk[:, F_VA:F_VA + 14, :].rearrange("p (o c) f -> p o c f", c=2)
        vjx, vjy = va[:, :, 0, :], va[:, :, 1, :]
        am = pk[:, F_AM:F_AM + MAX_NEI, :]
        cx, cy = pk[:, F_PC, :], pk[:, F_PC + 1, :]
        cvx, cvy = pk[:, F_VC, :], pk[:, F_VC + 1, :]
        cm = pk[:, F_CM, :]
        v = pk[:, F_V, :]

        # v2 = 2*v^2
        STT(out=self.v2[:], in0=v, scalar=2.0, in1=v, op0=OP.mult, op1=OP.mult)

        sA = self.prod[:, 0:10 * F].rearrange("p (o f) -> p o f", o=10)
        sB = self.prod[:, 10 * F:20 * F].rearrange("p (o f) -> p o f", o=10)

        # ---- obs rows (planes 0-9; W pairs 0..19) ----
        W0 = Wv[:, 0:20].rearrange("p (o c) f -> p o c f", c=2)[:, :, 0, :]
        W1 = Wv[:, 0:20].rearrange("p (o c) f -> p o c f", c=2)[:, :, 1, :]
        STT(out=W0, in0=lx, scalar=2.0 * K0, in1=om, op0=OP.mult, op1=OP.mult)
        STT(out=sA, in0=ly, scalar=2.0 * K0, in1=_bc(v, 10), op0=OP.mult, op1=OP.mult)
        TT(out=W1, in0=sA, in1=om, op=OP.mult)
        # h_rhs = 2v^2 - 6 lx v + 2 lx^2 + 2 ly^2 - 0.5
        STT(out=sA, in0=lx, scalar=-6.0, in1=_bc(v, 10), op0=OP.mult, op1=OP.mult)
        STT(out=sB, in0=lx, scalar=2.0, in1=lx, op0=OP.mult, op1=OP.mult)
        TT(out=sA, in0=sA, in1=sB, op=OP.add)
        STT(out=sB, in0=ly, scalar=2.0, in1=ly, op0=OP.mult, op1=OP.mult)
        TT(out=sA, in0=sA, in1=sB, op=OP.add)
        TT(out=sA, in0=sA, in1=_bc(self.v2[:], 10), op=OP.add)
        nc.vector.tensor_scalar(out=sA, in0=sA, scalar1=-0.5, scalar2=None, op0=OP.add)
        # mask: b = (h - BIG)*m + BIG
        STT(out=sA, in0=sA, scalar=-BIG, in1=om, op0=OP.add, op1=OP.mult)
        nc.vector.tensor_scalar(out=bv[:, 0:10], in0=sA, scalar1=BIG, scalar2=None, op0=OP.add)

        # ---- nei rows (planes 10-16; W pairs 20..33) ----
        sA7 = self.prod[:, 0:7 * F].rearrange("p (o f) -> p o f", o=7)
        sB7 = self.prod[:, 7 * F:14 * F].rearrange("p (o f) -> p o f", o=7)
        sC7 = self.prod[:, 14 * F:21 * F].rearrange("p (o f) -> p o f", o=7)
        W0 = Wv[:, 20:34].rearrange("p (o c) f -> p o c f", c=2)[:, :, 0, :]
        W1 = Wv[:, 20:34].rearrange("p (o c) f -> p o c f", c=2)[:, :, 1, :]
        STT(out=W0, in0=ax, scalar=2.0 * K0, in1=am, op0=OP.mult, op1=OP.mult)
        STT(out=sA7, in0=ay, scalar=2.0 * K0, in1=_bc(v, 7), op0=OP.mult, op1=OP.mult)
        STT(out=sB7, in0=ay, scalar=-2.0 * K0, in1=vjx, op0=OP.mult, op1=OP.mult)
        TT(out=sA7, in0=sA7, in1=sB7, op=OP.add)
        STT(out=sB7, in0=ax, scalar=2.0 * K0, in1=vjy, op0=OP.mult, op1=OP.mult)
        TT(out=sA7, in0=sA7, in1=sB7, op=OP.add)
        TT(out=W1, in0=sA7, in1=am, op=OP.mult)
        # h = 2v^2 - 4 v vjx + 2 vjx^2 + 2 vjy^2 - 6 ax v + 6 ax vjx + 6 ay vjy
        #     + 2 ax^2 + 2 ay^2 - 1.28
        STT(out=sA7, in0=vjx, scalar=-4.0, in1=_bc(v, 7), op0=OP.mult, op1=OP.mult)
        STT(out=sB7, in0=vjx, scalar=2.0, in1=vjx, op0=OP.mult, op1=OP.mult)
        TT(out=sA7, in0=sA7, in1=sB7, op=OP.add)
        STT(out=sB7, in0=vjy, scalar=2.0, in1=vjy, op0=OP.mult, op1=OP.mult)
        TT(out=sA7, in0=sA7, in1=sB7, op=OP.add)
        STT(out=sB7, in0=ax, scalar=-6.0, in1=_bc(v, 7), op0=OP.mult, op1=OP.mult)
        TT(out=sA7, in0=sA7, in1=sB7, op=OP.add)
        STT(out=sB7, in0=ax, scalar=6.0, in1=vjx, op0=OP.mult, op1=OP.mult)
        TT(out=sA7, in0=sA7, in1=sB7, op=OP.add)
        STT(out=sB7, in0=ay, scalar=6.0, in1=vjy, op0=OP.mult, op1=OP.mult)
        TT(out=sA7, in0=sA7, in1=sB7, op=OP.add)
        STT(out=sB7, in0=ax, scalar=2.0, in1=ax, op0=OP.mult, op1=OP.mult)
        TT(out=sA7, in0=sA7, in1=sB7, op=OP.add)
        STT(out=sB7, in0=ay, scalar=2.0, in1=ay, op0=OP.mult, op1=OP.mult)
        TT(out=sA7, in0=sA7, in1=sB7, op=OP.add)
        TT(out=sA7, in0=sA7, in1=_bc(self.v2[:], 7), op=OP.add)
        nc.vector.tensor_scalar(out=sA7, in0=sA7, scalar1=-1.28, scalar2=None, op0=OP.add)
        STT(out=sA7, in0=sA7, scalar=-BIG, in1=am, op0=OP.add, op1=OP.mult)
        nc.vector.tensor_scalar(out=bv[:, 10:17], in0=sA7, scalar1=BIG, scalar2=None, op0=OP.add)

        # ---- cn row (plane 17; W pair 34,35) ----
        s1 = _pl(self.prod, 0)
        s2 = _pl(self.prod, 1)
        STT(out=Wv[:, 34], in0=cx, scalar=-2.0 * K0, in1=cm, op0=OP.mult, op1=OP.mult)
        STT(out=s1, in0=cy, scalar=-2.0 * K0, in1=v, op0=OP.mult, op1=OP.mult)
        STT(out=s2, in0=cy, scalar=2.0 * K0, in1=cvx, op0=OP.mult, op1=OP.mult)
        TT(out=s1, in0=s1, in1=s2, op=OP.add)
        STT(out=s2, in0=cx, scalar=-2.0 * K0, in1=cvy, op0=OP.mult, op1=OP.mult)
        TT(out=s1, in0=s1, in1=s2, op=OP.add)
        TT(out=Wv[:, 35], in0=s1, in1=cm, op=OP.mult)
        # h = -2v^2 + 4 v cvx - 2 cvx^2 - 2 cvy^2 + 6 cx v - 6 cx cvx - 6 cy cvy
        #     - 2 cx^2 - 2 cy^2 + 50
        STT(out=s1, in0=cvx, scalar=4.0, in1=v, op0=OP.mult, op1=OP.mult)
        STT(out=s2, in0=cvx, scalar=-2.0, in1=cvx, op0=OP.mult, op1=OP.mult)
        TT(out=s1, in0=s1, in1=s2, op=OP.add)
        STT(out=s2, in0=cvy, scalar=-2.0, in1=cvy, op0=OP.mult, op1=OP.mult)
        TT(out=s1, in0=s1, in1=s2, op=OP.add)
        STT(out=s2, in0=cx, scalar=6.0, in1=v, op0=OP.mult, op1=OP.mult)
        TT(out=s1, in0=s1, in1=s2, op=OP.add)
        STT(out=s2, in0=cx, scalar=-6.0, in1=cvx, op0=OP.mult, op1=OP.mult)
        TT(out=s1, in0=s1, in1=s2, op=OP.add)
        STT(out=s2, in0=cy, scalar=-6.0, in1=cvy, op0=OP.mult, op1=OP.mult)
        TT(out=s1, in0=s1, in1=s2, op=OP.add)
        STT(out=s2, in0=cx, scalar=-2.0, in1=cx, op0=OP.mult, op1=OP.mult)
        TT(out=s1, in0=s1, in1=s2, op=OP.add)
        STT(out=s2, in0=cy, scalar=-2.0, in1=cy, op0=OP.mult, op1=OP.mult)
        TT(out=s1, in0=s1, in1=s2, op=OP.add)
        TT(out=s1, in0=s1, in1=self.v2[:], op=OP.subtract)
        nc.vector.tensor_scalar(out=s1, in0=s1, scalar1=50.0, scalar2=None, op0=OP.add)
        STT(out=s1, in0=s1, scalar=-BIG, in1=cm, op0=OP.add, op1=OP.mult)
        nc.vector.tensor_scalar(out=_pl(self.b, 17), in0=s1, scalar1=BIG, scalar2=None, op0=OP.add)

        # ---- slack/box b, power scale planes ----
        nc.vector.memset(_pl(self.b, 18, 3), 0.0)
        nc.vector.memset(_pl(self.b, 21, 2), 2.0)
        nc.vector.memset(_pl(self.b, 23, 2), 1.0)
        nc.vector.memset(_pl(self.SSp, 0, 2), 1.0)
        for c in range(3):
            nc.vector.memset(_pl(self.SSp, 2 + c), -PINV2)


    def emit_rsqrt(self, dst, src, newton=0):
        """dst = rsqrt(src) via reciprocal + ACT Sqrt seed + Newton steps."""
        nc = self.nc
        nc.vector.reciprocal(out=self.sc1[:], in_=src)
        a = nc.scalar.activation(dst, self.sc1[:], AF.Sqrt)
        self._act_fence([a])
        for _ in range(newton):
            nc.vector.tensor_tensor(out=self.sc1[:], in0=dst, in1=dst, op=OP.mult)
            nc.vector.tensor_tensor(out=self.sc1[:], in0=src, in1=self.sc1[:], op=OP.mult)
            nc.vector.tensor_scalar(out=self.sc1[:], in0=self.sc1[:],
                                    scalar1=-0.5, scalar2=1.5, op0=OP.mult, op1=OP.add)
            nc.vector.tensor_tensor(out=dst, in0=dst, in1=self.sc1[:], op=OP.mult)

    # ---------------- power iteration + step ----------------
    def power_phase(self, n_power):
        nc = self.nc
        TT = nc.vector.tensor_tensor
        nc.vector.memset(self.yh[:], 1.0)
        sq = self.prod[:, 0:25 * F]
        # Power iteration is scale-invariant, so intermediate normalizations
        # only bound the dynamic range. lambda_max <= ~2e3, so 4 unnormalized
        # steps grow |v| by <= ~1.3e13 and sum(v^2) stays < 4e27 << fp32 max.
        # Normalize every 4th step (and the last, which feeds the Rayleigh
        # quotient and gets Newton refinement) to skip ~3/4 of the sq/reduce/
        # rsqrt/scale chains.
        cur = self.yh
        for it in range(n_power):
            self.forward(cur, self.SSp, None, self.X)
            self.backward_T(self.X)
            if it % 4 == 3 or it == n_power - 1:
                TT(out=sq, in0=self.Tbuf[:], in1=self.Tbuf[:], op=OP.mult)
                nc.vector.tensor_reduce(
                    out=self.ns[:], in_=sq.rearrange("p (m f) -> p f m", m=25),
                    axis=AX.X, op=OP.add)
                self.emit_rsqrt(self.rn[:], self.ns[:],
                                newton=2 if it == n_power - 1 else 0)
                TT(out=_pv(self.yh, 25), in0=_pv(self.Tbuf, 25),
                   in1=_bc(self.rn[:], 25), op=OP.mult)
                cur = self.yh
            else:
                # next forward reads Tbuf directly; its z-reads all precede
                # the next backward's Tbuf writes, so reuse is safe.
                cur = self.Tbuf
        # Rayleigh L = v . (M v); then rs = rsqrt(L + 1e-6), s* = rs^2
        self.forward(self.yh, self.SSp, None, self.X)
        self.backward_T(self.X)
        TT(out=sq, in0=self.yh[:], in1=self.Tbuf[:], op=OP.mult)
        nc.vector.tensor_reduce(
            out=self.ns[:], in_=sq.rearrange("p (m f) -> p f m", m=25),
            axis=AX.X, op=OP.add)
        nc.vector.tensor_scalar(out=self.ns[:], in0=self.ns[:],
                                scalar1=1e-6, scalar2=None, op0=OP.add)
        self.emit_rsqrt(self.rs[:], self.ns[:], newton=2)
        TT(out=self.sstar[:], in0=self.rs[:], in1=self.rs[:], op=OP.mult)

    # ---------------- FISTA setup ----------------
    def fista_setup(self):
        nc = self.nc
        TT = nc.vector.tensor_tensor
        # btil = b * rs
        TT(out=_pv(self.btil, 25), in0=_pv(self.b, 25), in1=_bc(self.rs[:], 25),
           op=OP.mult)
        # SS = [-s*, -s*, PINV2*s* x3]
        a1 = nc.scalar.activation(
            _pl(self.SS, 0, 2).rearrange("p (c f) -> p c f", c=2),
            _bc(self.sstar[:], 2), AF.Copy, scale=-1.0)
        a2 = nc.scalar.activation(
            _pl(self.SS, 2, 3).rearrange("p (c f) -> p c f", c=3),
            _bc(self.sstar[:], 3), AF.Copy, scale=PINV2)
        self._act_fence([a1, a2])
        # Qadd = SS*FWD(btil) + rs * q~   (q~ = [sqrt2 u0, sqrt2 u1, 0,0,0])
        self.forward(self.btil, self.SS, None, self.Qadd)
        uap = self.pk[:, F_U:F_U + 2, :]
        nc.vector.scalar_tensor_tensor(
            out=self.u2[:].rearrange("p (c f) -> p c f", c=2),
            in0=uap, scalar=SQ2, in1=_bc(self.rs[:], 2), op0=OP.mult, op1=OP.mult)
        TT(out=_pl(self.Qadd, 0, 2), in0=_pl(self.Qadd, 0, 2), in1=self.u2[:],
           op=OP.add)
        # lam = yh = -btil
        a1 = nc.scalar.activation(self.lamA[:], self.btil[:], AF.Copy, scale=-1.0)
        a2 = nc.scalar.activation(self.yh[:], self.btil[:], AF.Copy, scale=-1.0)
        self._act_fence([a1, a2])

    # ---------------- FISTA loop ----------------
    def fista(self, n_fista):
        nc = self.nc
        betas = _betas(n_fista)
        lams = [self.lamA, self.lamB]
        # tp = -beta*lam_prev runs on the (otherwise idle) ACT engine. lam_prev
        # is ready at iteration start, so the ACT op overlaps the whole DVE
        # chain; the DVE only pays one fused STT for the momentum update:
        #   yh = (1+beta)*lam_new + tp
        # Rounding differs from (lam_new - lam_prev)*beta + lam_new by ~1 ulp,
        # which the harness tolerance absorbs.
        tps = [self.tpA, self.tpB]
        for it in range(n_fista):
            lam_prev = lams[it % 2]
            lam_new = lams[(it + 1) % 2]
            beta = betas[it]
            tp = tps[it % 2]
            a_tp = nc.scalar.activation(tp[:], lam_prev[:], AF.Copy,
                                        scale=-float(beta))
            self.forward(self.yh, self.SS, self.Qadd, self.X)
            self.backward_T(self.X)
            # arg = T + yh ; lam_new = max(arg,0) - btil
            nc.vector.tensor_tensor(out=self.Tbuf[:], in0=self.Tbuf[:],
                                    in1=self.yh[:], op=OP.add)
            nc.vector.scalar_tensor_tensor(
                out=lam_new[:], in0=self.Tbuf[:], scalar=0.0, in1=self.btil[:],
                op0=OP.max, op1=OP.subtract)
            # yh = (1+beta)*lam_new + tp
            self._act_fence([a_tp])
            nc.vector.scalar_tensor_tensor(
                out=self.yh[:], in0=lam_new[:], scalar=float(1.0 + beta),
                in1=tp[:], op0=OP.mult, op1=OP.add)
        return lams[n_fista % 2]

    # ---------------- finale ----------------
    def finale(self, lam_final, out_dram):
        nc = self.nc
        TT = nc.vector.tensor_tensor
        self.forward(lam_final, self.SS, self.Qadd, self.X)
        # u = K0 * X[0:2] / rs ; 1/rs = sqrt(L+1e-6) = ns * rs
        TT(out=self.sc1[:], in0=self.ns[:], in1=self.rs[:], op=OP.mult)
        a = nc.scalar.activation(self.sc2[:], self.sc1[:], AF.Copy, scale=K0)
        self.last_act = a
        self._act_fence([a])
        self.last_dve = TT(out=self.opack[:].rearrange("p (f c) -> p c f", c=2),
           in0=_pl(self.X, 0, 2).rearrange("p (c f) -> p c f", c=2),
           in1=_bc(self.sc2[:], 2), op=OP.mult)
        self.out_dma = nc.sync.dma_start(
            out=out_dram.ap().rearrange("(p f) c -> p (f c)", p=P),
            in_=self.opack[:])

    def terminals(self):
        return [self.in_dma, self.last_act, self.last_dve, self.out_dma]


def build_nc(n_power=N_POWER, n_fista=N_FISTA):
    nc = bass.Bass("TRN2")
    din = nc.dram_tensor("packed", [BPC, NFEAT], f32, kind="ExternalInput")
    dout = nc.dram_tensor("u_safe", [BPC, 2], f32, kind="ExternalOutput")

    with tile.TileContext(nc) as tc:
        with ExitStack() as ctx:
            em = Emit(ctx, tc)
            natpool = ctx.enter_context(tc.tile_pool(name="nat", bufs=1))
            natt = natpool.tile([P, F * NFEAT], f32, name="nat", tag="nat")
            # single SWDGE DMA -> one DMA semaphore for all downstream waits
            em.in_dma = nc.gpsimd.dma_start(
                out=natt[:], in_=din.ap().rearrange("(p f) a -> p (f a)", p=P))
            em.precompute(natt)
            em.power_phase(n_power)
            em.fista_setup()
            lam_final = em.fista(n_fista)
            em.finale(lam_final, dout)
            # Exit fence: the tile-exit drain would wait on every active proc
            # (ACT, DVE, DMA queues) at once, exceeding walrus's one-sync-wait
            # -per-instruction limit. Chain sync-engine NOPs, one dep each, so
            # the SP engine observes every proc before the drain.
            for ti in em.terminals():
                nop = nc.sync.nop()
                tile.add_dep_helper(nop.ins, ti.ins, sync=True,
                                    reason="exit fence")
    return nc


_NC_CACHE = {}


def _get_nc(n_power=N_POWER, n_fista=N_FISTA):
    key = (n_power, n_fista)
    if key not in _NC_CACHE:
        _NC_CACHE[key] = build_nc(n_power, n_fista)
    return _NC_CACHE[key]


def pack_inputs(inputs, lo, hi):
    """Pack the raw input dict (rows [lo, hi)) into one (n, NFEAT) array."""
    n = hi - lo
    cols = [np.asarray(inputs[name], np.float32)[lo:hi].reshape(n, -1)
            for name, _ in RAW_SPECS]
    return np.ascontiguousarray(np.concatenate(cols, axis=1))


def kernel(**inputs):
    """Full-input entry point: shard batch over 8 cores, run, gather."""
    nc = _get_nc()
    in_maps = [{"packed": pack_inputs(inputs, c * BPC, (c + 1) * BPC)}
               for c in range(NCORES)]
    res = run_bass_kernel_spmd(nc, in_maps, list(range(NCORES)))
    return np.concatenate([res.results[c]["u_safe"] for c in range(NCORES)],
                          axis=0)


if __name__ == "__main__":
    # smoke test on random data against a tiny numpy reference path
    rng = np.random.default_rng(0)
    demo = {
        "u_nominal": rng.standard_normal((B_FULL, 2)).astype(np.float32),
        "v_current": rng.uniform(0, 1, (B_FULL, 1)).astype(np.float32),
        "p_obs": (2 * rng.standard_normal((B_FULL, MAX_OBS, 2))).astype(np.float32),
        "obs_mask": np.ones((B_FULL, MAX_OBS), np.float32),
        "p_agents": (2 * rng.standard_normal((B_FULL, MAX_NEI, 2))).astype(np.float32),
        "v_agents_local": rng.standard_normal((B_FULL, MAX_NEI, 2)).astype(np.float32),
        "agents_mask": np.ones((B_FULL, MAX_NEI), np.float32),
        "p_c_agent": (2 * rng.standard_normal((B_FULL, 1, 2))).astype(np.float32),
        "v_c_agent": rng.standard_normal((B_FULL, 1, 2)).astype(np.float32),
        "closest_mask": np.ones((B_FULL, 1), np.float32),
    }
    out = kernel(**demo)
    print(out.shape, out.dtype, np.abs(out).max())

